# revision 18
# baseline (speedup 1.0000x reference)
"""Trainium2 Bass kernel for a pre-LN transformer block (attention + FFN).

Sharding over 8 NeuronCores: core (b, g) = batch b (0..3) x head-group g (0..1).
Each core runs LN1 + QKV (its 8 heads) + causal attention + its slice of the
output projection for its batch; a pairwise bf16 ReduceScatter (2 chunks,
first overlapped with attention of the second query half) sums the two
head-groups' partial attn_out; each core then finishes 1024 rows
(residual + LN2 + full FFN).

Attention inner loop: the two heads of a q/k pair live on partitions 0:64 and
64:128, so their K=64 score matmuls run CONCURRENTLY in the PE array via
row-group tiling (tile_position (0,0)/(64,0)); a 1-step software pipeline
issues scores(i)/exp(i) before ctx(i-1) so the PE never stalls on the scalar
engine and the HAM clock stays warm. Softmax denominators via a ones-column
on V and reciprocal_approx_fast.
"""
import sys

if "/opt/trn_rl_repo" not in sys.path:
    sys.path.insert(0, "/opt/trn_rl_repo")

import contextlib

import numpy as np
import ml_dtypes

import concourse.bass as bass
import concourse.bacc as bacc
import concourse.tile as tile
from concourse import mybir
from concourse.bass_utils import run_bass_kernel_spmd

F32 = mybir.dt.float32
F32R = mybir.dt.float32r
BF16 = mybir.dt.bfloat16
AF = mybir.ActivationFunctionType
OP = mybir.AluOpType

B, S, D, H = 4, 2048, 1024, 16
HD = D // H
FF = 4 * D
EPS = 1e-5
GH = 8          # heads per core
NP = 128        # partitions
SC = S // NP    # 16 seq chunks of 128
DC = D // NP    # 8 d-chunks
QC = S // 512   # 4 q-chunks of 512
FC = FF // NP   # 32 ff chunks of 128
RROWS = S // 2  # 1024 rows finalized per core


def _ln_stats(nc, pool, x_ap, eps_sb):
    """bn_stats/bn_aggr mean+rstd for a [128, D] fp32 tile. Returns mv tile;
    mv[:,0:1]=mean, mv[:,1:2]=rstd (after rsqrt)."""
    stats = pool.tile([NP, 2, 6], F32, tag="lnstats")
    nc.vector.bn_stats(out=stats[:, 0, :], in_=x_ap[:, 0:512])
    nc.vector.bn_stats(out=stats[:, 1, :], in_=x_ap[:, 512:1024])
    mv = pool.tile([NP, 2], F32, tag="lnmv")
    nc.vector.bn_aggr(out=mv[:], in_=stats[:])
    nc.scalar.activation(out=mv[:, 1:2], in_=mv[:, 1:2], func=AF.Sqrt,
                         bias=eps_sb[:], scale=1.0)
    nc.vector.reciprocal(out=mv[:, 1:2], in_=mv[:, 1:2])
    return mv


def build(apply_ln1_affine=False, apply_ln2_affine=False):
    nc = bacc.Bacc("TRN2", num_devices=8)

    # ---- DRAM parameters (per-core shards, laid out host-side) ----
    x_p = nc.declare_dram_parameter("x", [S, D], F32, isOutput=False)
    xres_p = nc.declare_dram_parameter("x_res", [RROWS, D], F32, isOutput=False)
    wq_p = nc.declare_dram_parameter("wq", [NP, DC, 512], BF16, isOutput=False)
    wk_p = nc.declare_dram_parameter("wk", [NP, DC, 512], BF16, isOutput=False)
    wv_p = nc.declare_dram_parameter("wv", [NP, DC, 512], BF16, isOutput=False)
    wproj_p = nc.declare_dram_parameter("wproj", [NP, 4, D], BF16, isOutput=False)
    wff1_p = nc.declare_dram_parameter("wff1", [FC, NP, DC * NP], BF16, isOutput=False)
    wff2_p = nc.declare_dram_parameter("wff2", [FC, NP, D], BF16, isOutput=False)
    bproj_p = nc.declare_dram_parameter("bproj", [1, D], F32, isOutput=False)
    bff1_p = nc.declare_dram_parameter("bff1", [NP, FC], F32, isOutput=False)
    bff2_p = nc.declare_dram_parameter("bff2", [1, D], F32, isOutput=False)
    ident_p = nc.declare_dram_parameter("ident", [NP, NP], BF16, isOutput=False)
    mask_p = nc.declare_dram_parameter("mask", [NP, NP], BF16, isOutput=False)
    if apply_ln1_affine:
        ln1w_p = nc.declare_dram_parameter("ln1w", [1, D], F32, isOutput=False)
        ln1b_p = nc.declare_dram_parameter("ln1b", [1, D], F32, isOutput=False)
    if apply_ln2_affine:
        ln2w_p = nc.declare_dram_parameter("ln2w", [1, D], F32, isOutput=False)
        ln2b_p = nc.declare_dram_parameter("ln2b", [1, D], F32, isOutput=False)
    out_p = nc.declare_dram_parameter("out", [RROWS, D], F32, isOutput=True)

    with tile.TileContext(nc) as tc:
        with contextlib.ExitStack() as stack:
            # ---------------- persistent pools ----------------
            const = stack.enter_context(tc.tile_pool(name="const", bufs=1))
            tp_ps = stack.enter_context(tc.tile_pool(name="tp_ps", bufs=2, space="PSUM"))
            dram = stack.enter_context(tc.tile_pool(name="dram", bufs=1, space="DRAM"))
            ctxT_pool = stack.enter_context(tc.tile_pool(name="ctxT", bufs=1))
            wp_pool = stack.enter_context(tc.tile_pool(name="wp", bufs=1))
            small = stack.enter_context(tc.tile_pool(name="small", bufs=2))

            eps_sb = const.tile([NP, 1], F32)
            nc.vector.memset(eps_sb[:], EPS)
            ones_f32 = const.tile([NP, 64], F32)
            nc.vector.memset(ones_f32[:], 1.0)
            ones_sb = const.tile([NP, 64], F32R)
            nc.vector.tensor_copy(out=ones_sb[:], in_=ones_f32[:])
            ident_sb = const.tile([NP, NP], BF16)
            nc.sync.dma_start(out=ident_sb[:], in_=ident_p[:, :])
            mask_sb = const.tile([NP, NP], BF16)
            nc.sync.dma_start(out=mask_sb[:], in_=mask_p[:, :])
            bproj_bc = const.tile([NP, D], F32)
            nc.gpsimd.dma_start(out=bproj_bc[:], in_=bproj_p[:, :].to_broadcast((NP, D)))
            bff2_bc = const.tile([NP, D], F32)
            nc.gpsimd.dma_start(out=bff2_bc[:], in_=bff2_p[:, :].to_broadcast((NP, D)))
            bff1_sb = const.tile([NP, FC], F32)
            nc.sync.dma_start(out=bff1_sb[:], in_=bff1_p[:, :])
            if apply_ln1_affine:
                ln1w_bc = const.tile([NP, D], F32)
                nc.gpsimd.dma_start(out=ln1w_bc[:], in_=ln1w_p[:, :].to_broadcast((NP, D)))
                ln1b_bc = const.tile([NP, D], F32)
                nc.gpsimd.dma_start(out=ln1b_bc[:], in_=ln1b_p[:, :].to_broadcast((NP, D)))
            if apply_ln2_affine:
                ln2w_bc = const.tile([NP, D], F32)
                nc.gpsimd.dma_start(out=ln2w_bc[:], in_=ln2w_p[:, :].to_broadcast((NP, D)))
                ln2b_bc = const.tile([NP, D], F32)
                nc.gpsimd.dma_start(out=ln2b_bc[:], in_=ln2b_p[:, :].to_broadcast((NP, D)))

            # bf16 collective buffers, 2 chunks of 1024 rows each
            cc_in0 = dram.tile([RROWS, D], BF16)
            cc_in1 = dram.tile([RROWS, D], BF16)
            cc_out0 = dram.tile([RROWS // 2, D], BF16)
            cc_out1 = dram.tile([RROWS // 2, D], BF16)

            ctxT = [ctxT_pool.tile([NP, S], BF16, tag=f"ctxT{p}", name=f"ctxT{p}") for p in range(4)]
            wp_sb = wp_pool.tile([NP, 4, D], BF16)

            # ---------------- attention-lifetime pools ----------------
            with contextlib.ExitStack() as att:
                qkT_pool = att.enter_context(tc.tile_pool(name="qkT", bufs=1))
                vaug_pool = att.enter_context(tc.tile_pool(name="vaug", bufs=1))
                e_pool = att.enter_context(tc.tile_pool(name="e", bufs=6))
                attn_pool = att.enter_context(tc.tile_pool(name="attn", bufs=2))
                sc_ps = att.enter_context(tc.tile_pool(name="sc_ps", bufs=4, space="PSUM"))

                vaug = [vaug_pool.tile([NP, GH, HD + 1], BF16, tag=f"v{sc}", name=f"vaug{sc}") for sc in range(SC)]
                for sc in range(SC):
                    nc.vector.memset(vaug[sc][:, :, HD:HD + 1], 1.0)
                qT = [qkT_pool.tile([NP, S], BF16, tag=f"qT{p}", name=f"qT{p}") for p in range(4)]
                kT = [qkT_pool.tile([NP, S], BF16, tag=f"kT{p}", name=f"kT{p}") for p in range(4)]

                # ---- prep phase: LN1 + hT, v rows, q/k projections ----
                with contextlib.ExitStack() as prep:
                    wqkv = prep.enter_context(tc.tile_pool(name="wqkv", bufs=1))
                    hT_pool = prep.enter_context(tc.tile_pool(name="hT", bufs=1))
                    xln = prep.enter_context(tc.tile_pool(name="xln", bufs=3))
                    hrow = prep.enter_context(tc.tile_pool(name="hrow", bufs=3))
                    lnst = prep.enter_context(tc.tile_pool(name="lnst", bufs=4))
                    qkv_ps = prep.enter_context(tc.tile_pool(name="qkv_ps", bufs=2, space="PSUM"))

                    wq_sb = wqkv.tile([NP, DC, 512], BF16)
                    wk_sb = wqkv.tile([NP, DC, 512], BF16)
                    wv_sb = wqkv.tile([NP, DC, 512], BF16)
                    nc.sync.dma_start(out=wq_sb[:], in_=wq_p[:, :, :])
                    nc.sync.dma_start(out=wk_sb[:], in_=wk_p[:, :, :])
                    nc.sync.dma_start(out=wv_sb[:], in_=wv_p[:, :, :])

                    hT = hT_pool.tile([NP, DC, S], BF16)        # LN1(x)^T

                    # LN1 + transpose h -> hT (copies on the scalar engine;
                    # the vector engine is the prep-phase bottleneck)
                    for sc in range(SC):
                        x_sb = xln.tile([NP, D], F32, tag="x")
                        # 4 column-stripe DMAs -> 4 queues in parallel (a single
                        # 512KB dma_start runs on one queue at ~22GB/s = 23us)
                        for st in range(4):
                            nc.sync.dma_start(
                                out=x_sb[:, st * 256:(st + 1) * 256],
                                in_=x_p[sc * NP:(sc + 1) * NP, st * 256:(st + 1) * 256])
                        mv = _ln_stats(nc, lnst, x_sb[:], eps_sb)
                        h_sb = hrow.tile([NP, D], BF16, tag="h")
                        nc.vector.tensor_scalar(
                            out=h_sb[:], in0=x_sb[:],
                            scalar1=mv[:, 0:1], scalar2=mv[:, 1:2],
                            op0=OP.subtract, op1=OP.mult)
                        if apply_ln1_affine:
                            nc.vector.tensor_mul(out=h_sb[:], in0=h_sb[:], in1=ln1w_bc[:])
                            nc.vector.tensor_add(out=h_sb[:], in0=h_sb[:], in1=ln1b_bc[:])
                        for dc in range(DC):
                            t_ps = tp_ps.tile([NP, NP], BF16, tag="tp")
                            nc.tensor.transpose(t_ps[:], h_sb[:, dc * NP:(dc + 1) * NP], ident_sb[:])
                            dst = hT[:, dc, sc * NP:(sc + 1) * NP]
                            if dc % 2 == 0:
                                nc.scalar.copy(out=dst, in_=t_ps[:])
                            else:
                                nc.vector.tensor_copy(out=dst, in_=t_ps[:])

                    # v rows: per seq chunk, out [128 seq, 512 all heads]
                    for sc in range(SC):
                        ps = qkv_ps.tile([NP, 512], F32, tag="qkvps")
                        for dc in range(DC):
                            nc.tensor.matmul(
                                ps[:],
                                hT[:, dc, sc * NP:(sc + 1) * NP],
                                wv_sb[:, dc, :],
                                start=(dc == 0), stop=(dc == DC - 1))
                        nc.vector.tensor_copy(
                            out=vaug[sc][:, :, 0:HD],
                            in_=ps[:].rearrange("p (h d) -> p h d", h=GH))

                    # q/k projections for ALL pairs
                    for hp in range(4):
                        for s4 in range(QC):
                            cols = slice(s4 * 512, (s4 + 1) * 512)
                            for w_sb, dst in ((wq_sb, qT[hp]), (wk_sb, kT[hp])):
                                ps = qkv_ps.tile([NP, 512], F32, tag="qkvps")
                                for dc in range(DC):
                                    nc.tensor.matmul(
                                        ps[:],
                                        w_sb[:, dc, hp * NP:(hp + 1) * NP],
                                        hT[:, dc, cols],
                                        start=(dc == 0), stop=(dc == DC - 1))
                                nc.vector.tensor_copy(out=dst[:, cols], in_=ps[:])

                # wproj load deferred out of the startup DMA window (first
                # needed by emit_proj at qc==1, ~100us later)
                nc.sync.dma_start(out=wp_sb[:], in_=wproj_p[:, :, :])

                # ---- attention: qc-major, both heads of a pair in flight ----
                with contextlib.ExitStack() as att2:
                    ctx_ps_pool = att2.enter_context(tc.tile_pool(name="ctx_ps", bufs=1, space="PSUM"))

                    def emit_proj(qs_lo, qs_hi, cc_dst):
                        for qs in range(qs_lo, qs_hi):
                            attn_sb = attn_pool.tile([NP, D], BF16, tag="attnsb")
                            for nch in range(2):
                                a_ps = sc_ps.tile([NP, 512], F32, tag="sc")
                                for pair in range(4):
                                    nc.tensor.matmul(
                                        a_ps[:],
                                        ctxT[pair][:, qs * NP:(qs + 1) * NP],
                                        wp_sb[:, pair, nch * 512:(nch + 1) * 512],
                                        start=(pair == 0), stop=(pair == 3))
                                nc.vector.tensor_copy(out=attn_sb[:, nch * 512:(nch + 1) * 512], in_=a_ps[:])
                            r = qs - qs_lo
                            nc.sync.dma_start(out=cc_dst[r * NP:(r + 1) * NP, :], in_=attn_sb[:])

                    pending_norm = []

                    def flush_norm():
                        while pending_norm:
                            pending_norm.pop(0)()

                    def make_norm(ctx_ps, hp, po, qbase):
                        def emit():
                            den = small.tile([NP, 512], F32R, tag="den", name="den")
                            nc.vector.tensor_copy(out=den[64:65, :], in_=ctx_ps[64:65, :])
                            b_ps = sc_ps.tile([64, 512], F32, tag="sc", name="b_ps")
                            nc.tensor.matmul(b_ps[:], ones_sb[64:65, :], den[64:65, :],
                                             start=True, stop=True)
                            b_sb = small.tile([64, 512], F32, tag="bsb", name="b_sb")
                            nc.vector.reciprocal_approx_fast(out=b_sb[:], in_=b_ps[:])
                            nc.vector.tensor_mul(
                                out=ctxT[hp][po:po + 64, qbase:qbase + 512],
                                in0=ctx_ps[0:64, :], in1=b_sb[:])
                        return emit

                    for qc in range(QC):
                        qbase = qc * 512
                        kcs = [4 * qc] + list(range(0, 4 * qc)) + [4 * qc + 1, 4 * qc + 2, 4 * qc + 3]
                        for hp in range(4):
                            ctxA = ctx_ps_pool.tile([HD + 1, 512], F32, tag="ctxA", name="ctxA")
                            ctxB = ctx_ps_pool.tile([HD + 1, 512], F32, tag="ctxB", name="ctxB")
                            pend = None
                            for i, kc in enumerate(kcs):
                                off = max(0, 128 * kc - qbase)
                                sA = sc_ps.tile([NP, 512], F32, tag="sc", name="sA")
                                sB = sc_ps.tile([NP, 512], F32, tag="sc", name="sB")
                                # concurrent in the PE array: row groups 0-63 / 64-127
                                nc.tensor.matmul(
                                    sA[:, off:512],
                                    kT[hp][0:64, kc * NP:(kc + 1) * NP],
                                    qT[hp][0:64, qbase + off:qbase + 512],
                                    start=True, stop=True)
                                nc.tensor.matmul(
                                    sB[:, off:512],
                                    kT[hp][64:128, kc * NP:(kc + 1) * NP],
                                    qT[hp][64:128, qbase + off:qbase + 512],
                                    start=True, stop=True)
                                eA = e_pool.tile([NP, 512], BF16, tag="esb", name="eA")
                                eB = e_pool.tile([NP, 512], BF16, tag="esb", name="eB")
                                nc.scalar.activation(out=eA[:, off:512], in_=sA[:, off:512], func=AF.Exp)
                                nc.scalar.activation(out=eB[:, off:512], in_=sB[:, off:512], func=AF.Exp)
                                if 4 * qc <= kc:
                                    nc.vector.tensor_mul(
                                        out=eA[:, off:off + 128], in0=eA[:, off:off + 128], in1=mask_sb[:])
                                    nc.vector.tensor_mul(
                                        out=eB[:, off:off + 128], in0=eB[:, off:off + 128], in1=mask_sb[:])
                                if i == 0:
                                    # previous pair's softmax normalization, deferred
                                    # past this pair's first scores/exps so the
                                    # scalar-engine exp stream never stalls at the
                                    # pair boundary (must drain before this pair's
                                    # first ctx matmul reuses the ctx banks)
                                    flush_norm()
                                if pend is not None:
                                    off_, eA_, eB_, kc_, first_ = pend
                                    nc.tensor.matmul(
                                        ctxA[:, off_:512], vaug[kc_][:, 2 * hp, :],
                                        eA_[:, off_:512], start=first_, stop=False)
                                    nc.tensor.matmul(
                                        ctxB[:, off_:512], vaug[kc_][:, 2 * hp + 1, :],
                                        eB_[:, off_:512], start=first_, stop=False)
                                pend = (off, eA, eB, kc, i == 0)
                            off_, eA_, eB_, kc_, first_ = pend
                            nc.tensor.matmul(
                                ctxA[:, off_:512], vaug[kc_][:, 2 * hp, :],
                                eA_[:, off_:512], start=first_, stop=True)
                            nc.tensor.matmul(
                                ctxB[:, off_:512], vaug[kc_][:, 2 * hp + 1, :],
                                eB_[:, off_:512], start=first_, stop=True)
                            pending_norm.append(make_norm(ctxA, hp, 0, qbase))
                            pending_norm.append(make_norm(ctxB, hp, 64, qbase))

                        if qc == 1:
                            flush_norm()
                            emit_proj(0, 8, cc_in0)
                            nc.gpsimd.collective_compute(
                                "ReduceScatter", OP.add,
                                replica_groups=[[0, 1], [2, 3], [4, 5], [6, 7]],
                                ins=[cc_in0[:].opt()], outs=[cc_out0[:].opt()])
                        if qc == 3:
                            flush_norm()
                            emit_proj(8, 16, cc_in1)

            # RS#2 issued OUTSIDE the attention pool scopes: the pool-stack
            # close drains all engines, so a collective issued inside would
            # serialize the whole FFN behind its completion.
            nc.gpsimd.collective_compute(
                "ReduceScatter", OP.add,
                replica_groups=[[0, 1], [2, 3], [4, 5], [6, 7]],
                ins=[cc_in1[:].opt()], outs=[cc_out1[:].opt()])

            # ---------------- FFN phase (1024 rows per core) ----------------
            with contextlib.ExitStack() as ffn:
                y_pool = ffn.enter_context(tc.tile_pool(name="y", bufs=1))
                y2T_pool = ffn.enter_context(tc.tile_pool(name="y2T", bufs=1))
                g_pool = ffn.enter_context(tc.tile_pool(name="g", bufs=64))
                yw = ffn.enter_context(tc.tile_pool(name="yw", bufs=3))
                lnst2 = ffn.enter_context(tc.tile_pool(name="lnst2", bufs=4))
                w1_pool = ffn.enter_context(tc.tile_pool(name="w1", bufs=5))
                w2_pool = ffn.enter_context(tc.tile_pool(name="w2", bufs=5))
                out_pool = ffn.enter_context(tc.tile_pool(name="outp", bufs=3))
                ff_ps_pool = ffn.enter_context(tc.tile_pool(name="ff_ps", bufs=2, space="PSUM"))
                z_ps_pool = ffn.enter_context(tc.tile_pool(name="z_ps", bufs=1, space="PSUM"))

                y = [y_pool.tile([NP, D], F32, tag=f"y{rc}", name=f"y{rc}") for rc in range(8)]
                y2T = y2T_pool.tile([NP, DC, RROWS], BF16)

                cc_outs = (cc_out0, cc_out1)
                for half in range(2):
                    # residual + LN2 for this half only, so half 0's FFN can
                    # run while the second ReduceScatter is still in flight
                    for r4 in range(4):
                        rc = half * 4 + r4
                        rs_sb = yw.tile([NP, D], BF16, tag="rs")
                        nc.sync.dma_start(out=rs_sb[:], in_=cc_outs[half][r4 * NP:(r4 + 1) * NP, :])
                        xr_sb = yw.tile([NP, D], F32, tag="xr")
                        for st in range(2):
                            nc.sync.dma_start(
                                out=xr_sb[:, st * 512:(st + 1) * 512],
                                in_=xres_p[rc * NP:(rc + 1) * NP, st * 512:(st + 1) * 512])
                        nc.vector.tensor_copy(out=y[rc][:], in_=rs_sb[:])
                        nc.vector.tensor_add(out=y[rc][:], in0=y[rc][:], in1=xr_sb[:])
                        nc.vector.tensor_add(out=y[rc][:], in0=y[rc][:], in1=bproj_bc[:])
                        mv = _ln_stats(nc, lnst2, y[rc][:], eps_sb)
                        y2_sb = yw.tile([NP, D], BF16, tag="y2")
                        nc.vector.tensor_scalar(
                            out=y2_sb[:], in0=y[rc][:],
                            scalar1=mv[:, 0:1], scalar2=mv[:, 1:2],
                            op0=OP.subtract, op1=OP.mult)
                        if apply_ln2_affine:
                            nc.vector.tensor_mul(out=y2_sb[:], in0=y2_sb[:], in1=ln2w_bc[:])
                            nc.vector.tensor_add(out=y2_sb[:], in0=y2_sb[:], in1=ln2b_bc[:])
                        for dc in range(DC):
                            t_ps = tp_ps.tile([NP, NP], BF16, tag="tp")
                            nc.tensor.transpose(t_ps[:], y2_sb[:, dc * NP:(dc + 1) * NP], ident_sb[:])
                            nc.scalar.copy(out=y2T[:, dc, rc * NP:(rc + 1) * NP], in_=t_ps[:])

                    hcols = slice(half * 512, (half + 1) * 512)
                    gts = []
                    for ffc in range(FC):
                        w1_sb = w1_pool.tile([NP, DC * NP], BF16, tag="w1")
                        nc.sync.dma_start(out=w1_sb[:], in_=wff1_p[ffc, :, :])
                        ff_ps = ff_ps_pool.tile([NP, 512], F32, tag="ffps")
                        for dc in range(DC):
                            nc.tensor.matmul(
                                ff_ps[:],
                                w1_sb[:, dc * NP:(dc + 1) * NP],
                                y2T[:, dc, hcols],
                                start=(dc == 0), stop=(dc == DC - 1))
                        g_sb = g_pool.tile([NP, 512], BF16, tag="g")
                        nc.scalar.activation(out=g_sb[:], in_=ff_ps[:], func=AF.Gelu,
                                             bias=bff1_sb[:, ffc:ffc + 1], scale=1.0)
                        gts.append(g_sb)
                    for nch in range(2):
                        ncols = slice(nch * 512, (nch + 1) * 512)
                        z_pss = [z_ps_pool.tile([NP, 512], F32, tag=f"zps{r}", name=f"zps{r}") for r in range(4)]
                        for ffc in range(FC):
                            w2_sb = w2_pool.tile([NP, 512], BF16, tag="w2")
                            nc.sync.dma_start(out=w2_sb[:], in_=wff2_p[ffc, :, ncols])
                            for r4 in range(4):
                                nc.tensor.matmul(
                                    z_pss[r4][:],
                                    gts[ffc][:, r4 * NP:(r4 + 1) * NP],
                                    w2_sb[:],
                                    start=(ffc == 0), stop=(ffc == FC - 1))
                        for r4 in range(4):
                            rc = half * 4 + r4
                            o_sb = out_pool.tile([NP, 512], F32, tag="osb")
                            nc.vector.tensor_add(out=o_sb[:], in0=z_pss[r4][:], in1=y[rc][:, ncols])
                            nc.vector.tensor_add(out=o_sb[:], in0=o_sb[:], in1=bff2_bc[:, ncols])
                            nc.sync.dma_start(out=out_p[rc * NP:(rc + 1) * NP, ncols], in_=o_sb[:])

    nc.compile()
    return nc


# ------------------------- host-side driver -------------------------

_BF = ml_dtypes.bfloat16


def _core_rows(g):
    return np.r_[512 * g:512 * g + 512, 1024 + 512 * g:1536 + 512 * g]


def _prep_core_inputs(inputs, b, g):
    x = np.asarray(inputs["x"], np.float32)
    w_qkv = np.asarray(inputs["w_qkv"], np.float32).reshape(D, H, HD, 3)
    hs = slice(g * GH, (g + 1) * GH)
    w_k = w_qkv[:, hs, :, 0].reshape(D, GH * HD)
    w_q = (w_qkv[:, hs, :, 1] * (HD ** -0.5)).reshape(D, GH * HD)
    w_v = w_qkv[:, hs, :, 2].reshape(D, GH * HD)

    def tile_kxm(w):  # [D, 512] -> [128, DC, 512]
        return np.ascontiguousarray(w.reshape(DC, NP, GH * HD).transpose(1, 0, 2))

    w_proj = np.asarray(inputs["w_proj"], np.float32)
    wp = np.ascontiguousarray(
        w_proj[g * 512:(g + 1) * 512, :].reshape(4, NP, D).transpose(1, 0, 2))

    w_ff1 = np.asarray(inputs["w_ff1"], np.float32)
    w1t = np.ascontiguousarray(
        w_ff1.reshape(DC, NP, FC, NP).transpose(2, 1, 0, 3).reshape(FC, NP, DC * NP))
    w_ff2 = np.asarray(inputs["w_ff2"], np.float32)
    w2t = np.ascontiguousarray(w_ff2.reshape(FC, NP, D))

    j = np.arange(NP)[:, None]
    i = np.arange(NP)[None, :]
    mask = (j <= i).astype(np.float32)

    return {
        "x": np.ascontiguousarray(x[b]),
        "x_res": np.ascontiguousarray(x[b][_core_rows(g)]),
        "wq": tile_kxm(w_q).astype(_BF),
        "wk": tile_kxm(w_k).astype(_BF),
        "wv": tile_kxm(w_v).astype(_BF),
        "wproj": wp.astype(_BF),
        "wff1": w1t.astype(_BF),
        "wff2": w2t.astype(_BF),
        "bproj": np.asarray(inputs["b_proj"], np.float32).reshape(1, D).copy(),
        "bff1": np.ascontiguousarray(np.asarray(inputs["b_ff1"], np.float32).reshape(FC, NP).T),
        "bff2": np.asarray(inputs["b_ff2"], np.float32).reshape(1, D).copy(),
        "ident": np.eye(NP, dtype=_BF),
        "mask": mask.astype(_BF),
    }


_NC_CACHE = {}


def kernel(**inputs):
    ln1w = np.asarray(inputs["ln1_w"], np.float32)
    ln1b = np.asarray(inputs["ln1_b"], np.float32)
    ln2w = np.asarray(inputs["ln2_w"], np.float32)
    ln2b = np.asarray(inputs["ln2_b"], np.float32)
    a1 = not (np.allclose(ln1w, 1.0) and np.allclose(ln1b, 0.0))
    a2 = not (np.allclose(ln2w, 1.0) and np.allclose(ln2b, 0.0))

    key = (a1, a2)
    if key not in _NC_CACHE:
        _NC_CACHE[key] = build(apply_ln1_affine=a1, apply_ln2_affine=a2)
    nc = _NC_CACHE[key]

    in_maps = []
    for core in range(8):
        b, g = core // 2, core % 2
        m = _prep_core_inputs(inputs, b, g)
        if a1:
            m["ln1w"] = ln1w.reshape(1, D).copy()
            m["ln1b"] = ln1b.reshape(1, D).copy()
        if a2:
            m["ln2w"] = ln2w.reshape(1, D).copy()
            m["ln2b"] = ln2b.reshape(1, D).copy()
        in_maps.append(m)

    res = run_bass_kernel_spmd(nc, in_maps, core_ids=list(range(8)))

    out = np.empty((B, S, D), np.float32)
    for core in range(8):
        b, g = core // 2, core % 2
        out[b][_core_rows(g)] = res.results[core]["out"]
    return out


# revision 22
# speedup vs baseline: 1.0379x; 1.0379x over previous
"""Trainium2 Bass kernel for a pre-LN transformer block (attention + FFN).

Sharding over 8 NeuronCores: core (b, g) = batch b (0..3) x head-group g (0..1).
Each core runs LN1 + QKV (its 8 heads) + causal attention + its slice of the
output projection for its batch; a pairwise bf16 ReduceScatter (2 chunks,
first overlapped with attention of the second query half) sums the two
head-groups' partial attn_out; each core then finishes 1024 rows
(residual + LN2 + full FFN).

Attention inner loop: the two heads of a q/k pair live on partitions 0:64 and
64:128, so their K=64 score matmuls run CONCURRENTLY in the PE array via
row-group tiling (tile_position (0,0)/(64,0)); a 1-step software pipeline
issues scores(i)/exp(i) before ctx(i-1) so the PE never stalls on the scalar
engine and the HAM clock stays warm. Softmax denominators via a ones-column
on V and reciprocal_approx_fast.
"""
import sys

if "/opt/trn_rl_repo" not in sys.path:
    sys.path.insert(0, "/opt/trn_rl_repo")

import contextlib

import numpy as np
import ml_dtypes

import concourse.bass as bass
import concourse.bacc as bacc
import concourse.tile as tile
from concourse import mybir
from concourse.bass_utils import run_bass_kernel_spmd

F32 = mybir.dt.float32
F32R = mybir.dt.float32r
BF16 = mybir.dt.bfloat16
AF = mybir.ActivationFunctionType
OP = mybir.AluOpType

B, S, D, H = 4, 2048, 1024, 16
HD = D // H
FF = 4 * D
EPS = 1e-5
GH = 8          # heads per core
NP = 128        # partitions
SC = S // NP    # 16 seq chunks of 128
DC = D // NP    # 8 d-chunks
QC = S // 512   # 4 q-chunks of 512
FC = FF // NP   # 32 ff chunks of 128
RROWS = S // 2  # 1024 rows finalized per core


def _ln_stats(nc, pool, x_ap, eps_sb):
    """bn_stats/bn_aggr mean+rstd for a [128, D] fp32 tile. Returns mv tile;
    mv[:,0:1]=mean, mv[:,1:2]=rstd (after rsqrt)."""
    stats = pool.tile([NP, 2, 6], F32, tag="lnstats")
    nc.vector.bn_stats(out=stats[:, 0, :], in_=x_ap[:, 0:512])
    nc.vector.bn_stats(out=stats[:, 1, :], in_=x_ap[:, 512:1024])
    mv = pool.tile([NP, 2], F32, tag="lnmv")
    nc.vector.bn_aggr(out=mv[:], in_=stats[:])
    nc.scalar.activation(out=mv[:, 1:2], in_=mv[:, 1:2], func=AF.Sqrt,
                         bias=eps_sb[:], scale=1.0)
    nc.vector.reciprocal(out=mv[:, 1:2], in_=mv[:, 1:2])
    return mv


def build(apply_ln1_affine=False, apply_ln2_affine=False):
    nc = bacc.Bacc("TRN2", num_devices=8)

    # ---- DRAM parameters (per-core shards, laid out host-side) ----
    x_p = nc.declare_dram_parameter("x", [S, D], F32, isOutput=False)
    xres_p = nc.declare_dram_parameter("x_res", [RROWS, D], F32, isOutput=False)
    wq_p = nc.declare_dram_parameter("wq", [NP, DC, 512], BF16, isOutput=False)
    wk_p = nc.declare_dram_parameter("wk", [NP, DC, 512], BF16, isOutput=False)
    wv_p = nc.declare_dram_parameter("wv", [NP, DC, 512], BF16, isOutput=False)
    wproj_p = nc.declare_dram_parameter("wproj", [NP, 4, D], BF16, isOutput=False)
    wff1_p = nc.declare_dram_parameter("wff1", [FC, NP, DC * NP], BF16, isOutput=False)
    wff2_p = nc.declare_dram_parameter("wff2", [FC, NP, D], BF16, isOutput=False)
    bproj_p = nc.declare_dram_parameter("bproj", [1, D], F32, isOutput=False)
    bff1_p = nc.declare_dram_parameter("bff1", [NP, FC], F32, isOutput=False)
    bff2_p = nc.declare_dram_parameter("bff2", [1, D], F32, isOutput=False)
    ident_p = nc.declare_dram_parameter("ident", [NP, NP], BF16, isOutput=False)
    mask_p = nc.declare_dram_parameter("mask", [NP, NP], BF16, isOutput=False)
    if apply_ln1_affine:
        ln1w_p = nc.declare_dram_parameter("ln1w", [1, D], F32, isOutput=False)
        ln1b_p = nc.declare_dram_parameter("ln1b", [1, D], F32, isOutput=False)
    if apply_ln2_affine:
        ln2w_p = nc.declare_dram_parameter("ln2w", [1, D], F32, isOutput=False)
        ln2b_p = nc.declare_dram_parameter("ln2b", [1, D], F32, isOutput=False)
    out_p = nc.declare_dram_parameter("out", [RROWS, D], F32, isOutput=True)

    with tile.TileContext(nc) as tc:
        with contextlib.ExitStack() as stack:
            # ---------------- persistent pools ----------------
            const = stack.enter_context(tc.tile_pool(name="const", bufs=1))
            tp_ps = stack.enter_context(tc.tile_pool(name="tp_ps", bufs=2, space="PSUM"))
            dram = stack.enter_context(tc.tile_pool(name="dram", bufs=1, space="DRAM"))
            ctxT_pool = stack.enter_context(tc.tile_pool(name="ctxT", bufs=1))
            wp_pool = stack.enter_context(tc.tile_pool(name="wp", bufs=1))
            small = stack.enter_context(tc.tile_pool(name="small", bufs=2))

            eps_sb = const.tile([NP, 1], F32)
            nc.vector.memset(eps_sb[:], EPS)
            ones_f32 = const.tile([NP, 64], F32)
            nc.vector.memset(ones_f32[:], 1.0)
            ones_sb = const.tile([NP, 64], F32R)
            nc.vector.tensor_copy(out=ones_sb[:], in_=ones_f32[:])
            ident_sb = const.tile([NP, NP], BF16)
            nc.sync.dma_start(out=ident_sb[:], in_=ident_p[:, :])
            mask_sb = const.tile([NP, NP], BF16)
            nc.sync.dma_start(out=mask_sb[:], in_=mask_p[:, :])
            bproj_bc = const.tile([NP, D], F32)
            nc.gpsimd.dma_start(out=bproj_bc[:], in_=bproj_p[:, :].to_broadcast((NP, D)))
            bff2_bc = const.tile([NP, D], F32)
            nc.gpsimd.dma_start(out=bff2_bc[:], in_=bff2_p[:, :].to_broadcast((NP, D)))
            bff1_sb = const.tile([NP, FC], F32)
            nc.sync.dma_start(out=bff1_sb[:], in_=bff1_p[:, :])
            if apply_ln1_affine:
                ln1w_bc = const.tile([NP, D], F32)
                nc.gpsimd.dma_start(out=ln1w_bc[:], in_=ln1w_p[:, :].to_broadcast((NP, D)))
                ln1b_bc = const.tile([NP, D], F32)
                nc.gpsimd.dma_start(out=ln1b_bc[:], in_=ln1b_p[:, :].to_broadcast((NP, D)))
            if apply_ln2_affine:
                ln2w_bc = const.tile([NP, D], F32)
                nc.gpsimd.dma_start(out=ln2w_bc[:], in_=ln2w_p[:, :].to_broadcast((NP, D)))
                ln2b_bc = const.tile([NP, D], F32)
                nc.gpsimd.dma_start(out=ln2b_bc[:], in_=ln2b_p[:, :].to_broadcast((NP, D)))

            # bf16 collective buffers, 2 chunks of 1024 rows each
            cc_in0 = dram.tile([RROWS, D], BF16)
            cc_in1 = dram.tile([RROWS, D], BF16)
            cc_out0 = dram.tile([RROWS // 2, D], BF16)
            cc_out1 = dram.tile([RROWS // 2, D], BF16)

            ctxT = [ctxT_pool.tile([NP, S], BF16, tag=f"ctxT{p}", name=f"ctxT{p}") for p in range(4)]
            wp_sb = wp_pool.tile([NP, 4, D], BF16)

            # ---------------- attention-lifetime pools ----------------
            with contextlib.ExitStack() as att:
                qkT_pool = att.enter_context(tc.tile_pool(name="qkT", bufs=1))
                vaug_pool = att.enter_context(tc.tile_pool(name="vaug", bufs=1))
                e_pool = att.enter_context(tc.tile_pool(name="e", bufs=6))
                attn_pool = att.enter_context(tc.tile_pool(name="attn", bufs=2))
                sc_ps = att.enter_context(tc.tile_pool(name="sc_ps", bufs=4, space="PSUM"))

                vaug = [vaug_pool.tile([NP, GH, HD + 1], BF16, tag=f"v{sc}", name=f"vaug{sc}") for sc in range(SC)]
                for sc in range(SC):
                    nc.vector.memset(vaug[sc][:, :, HD:HD + 1], 1.0)
                qT = [qkT_pool.tile([NP, S], BF16, tag=f"qT{p}", name=f"qT{p}") for p in range(4)]
                kT = [qkT_pool.tile([NP, S], BF16, tag=f"kT{p}", name=f"kT{p}") for p in range(4)]

                # ---- prep phase: LN1 + hT, v rows, q/k projections ----
                with contextlib.ExitStack() as prep:
                    wqkv = prep.enter_context(tc.tile_pool(name="wqkv", bufs=1))
                    hT_pool = prep.enter_context(tc.tile_pool(name="hT", bufs=1))
                    xln = prep.enter_context(tc.tile_pool(name="xln", bufs=2))
                    hrow = prep.enter_context(tc.tile_pool(name="hrow", bufs=3))
                    lnst = prep.enter_context(tc.tile_pool(name="lnst", bufs=4))
                    qkv_ps = prep.enter_context(tc.tile_pool(name="qkv_ps", bufs=2, space="PSUM"))

                    wq_sb = wqkv.tile([NP, DC, 512], BF16)
                    wk_sb = wqkv.tile([NP, DC, 512], BF16)
                    wv_sb = wqkv.tile([NP, DC, 512], BF16)
                    nc.sync.dma_start(out=wq_sb[:], in_=wq_p[:, :, :])
                    nc.sync.dma_start(out=wk_sb[:], in_=wk_p[:, :, :])
                    nc.sync.dma_start(out=wv_sb[:], in_=wv_p[:, :, :])

                    hT = hT_pool.tile([NP, DC, S], BF16)        # LN1(x)^T

                    # LN1 + transpose h -> hT (copies on the scalar engine;
                    # the vector engine is the prep-phase bottleneck)
                    for sc in range(SC):
                        x_sb = xln.tile([NP, D], F32, tag="x")
                        nc.sync.dma_start(out=x_sb[:], in_=x_p[sc * NP:(sc + 1) * NP, :])
                        mv = _ln_stats(nc, lnst, x_sb[:], eps_sb)
                        h_sb = hrow.tile([NP, D], BF16, tag="h")
                        nc.vector.tensor_scalar(
                            out=h_sb[:], in0=x_sb[:],
                            scalar1=mv[:, 0:1], scalar2=mv[:, 1:2],
                            op0=OP.subtract, op1=OP.mult)
                        if apply_ln1_affine:
                            nc.vector.tensor_mul(out=h_sb[:], in0=h_sb[:], in1=ln1w_bc[:])
                            nc.vector.tensor_add(out=h_sb[:], in0=h_sb[:], in1=ln1b_bc[:])
                        for dc in range(DC):
                            t_ps = tp_ps.tile([NP, NP], BF16, tag="tp")
                            nc.tensor.transpose(t_ps[:], h_sb[:, dc * NP:(dc + 1) * NP], ident_sb[:])
                            dst = hT[:, dc, sc * NP:(sc + 1) * NP]
                            if dc % 2 == 0:
                                nc.scalar.copy(out=dst, in_=t_ps[:])
                            else:
                                nc.vector.tensor_copy(out=dst, in_=t_ps[:])

                    # v rows: per seq chunk, out [128 seq, 512 all heads]
                    for sc in range(SC):
                        ps = qkv_ps.tile([NP, 512], F32, tag="qkvps")
                        for dc in range(DC):
                            nc.tensor.matmul(
                                ps[:],
                                hT[:, dc, sc * NP:(sc + 1) * NP],
                                wv_sb[:, dc, :],
                                start=(dc == 0), stop=(dc == DC - 1))
                        nc.vector.tensor_copy(
                            out=vaug[sc][:, :, 0:HD],
                            in_=ps[:].rearrange("p (h d) -> p h d", h=GH))

                    # q/k projections for ALL pairs
                    for hp in range(4):
                        for s4 in range(QC):
                            cols = slice(s4 * 512, (s4 + 1) * 512)
                            for w_sb, dst in ((wq_sb, qT[hp]), (wk_sb, kT[hp])):
                                ps = qkv_ps.tile([NP, 512], F32, tag="qkvps")
                                for dc in range(DC):
                                    nc.tensor.matmul(
                                        ps[:],
                                        w_sb[:, dc, hp * NP:(hp + 1) * NP],
                                        hT[:, dc, cols],
                                        start=(dc == 0), stop=(dc == DC - 1))
                                nc.vector.tensor_copy(out=dst[:, cols], in_=ps[:])

                # wproj load deferred out of the startup DMA window (first
                # needed by emit_proj at qc==1, ~100us later)
                nc.sync.dma_start(out=wp_sb[:], in_=wproj_p[:, :, :])

                # ---- attention: qc-major, both heads of a pair in flight ----
                with contextlib.ExitStack() as att2:
                    ctx_ps_pool = att2.enter_context(tc.tile_pool(name="ctx_ps", bufs=1, space="PSUM"))

                    def emit_one_proj(qs, cc_dst, r):
                        attn_sb = attn_pool.tile([NP, D], BF16, tag="attnsb", name="attn_sb")
                        for nch in range(2):
                            a_ps = sc_ps.tile([NP, 512], F32, tag="sc", name="a_ps")
                            for pair in range(4):
                                nc.tensor.matmul(
                                    a_ps[:],
                                    ctxT[pair][:, qs * NP:(qs + 1) * NP],
                                    wp_sb[:, pair, nch * 512:(nch + 1) * 512],
                                    start=(pair == 0), stop=(pair == 3))
                            nc.vector.tensor_copy(out=attn_sb[:, nch * 512:(nch + 1) * 512], in_=a_ps[:])
                        nc.sync.dma_start(out=cc_dst[r * NP:(r + 1) * NP, :], in_=attn_sb[:])

                    def emit_proj(qs_lo, qs_hi, cc_dst):
                        for qs in range(qs_lo, qs_hi):
                            emit_one_proj(qs, cc_dst, qs - qs_lo)

                    # proj chains for rows 0-1023, interleaved one-per-step
                    # into qc==2's attention so the scalar exp stream is
                    # never starved by a block of PE-only proj work
                    pending_proj = []

                    pending_norm = []

                    def flush_norm():
                        while pending_norm:
                            pending_norm.pop(0)()

                    def make_norm(ctx_ps, hp, po, qbase):
                        def emit():
                            den = small.tile([NP, 512], F32R, tag="den", name="den")
                            nc.vector.tensor_copy(out=den[64:65, :], in_=ctx_ps[64:65, :])
                            b_ps = sc_ps.tile([64, 512], F32, tag="sc", name="b_ps")
                            nc.tensor.matmul(b_ps[:], ones_sb[64:65, :], den[64:65, :],
                                             start=True, stop=True)
                            b_sb = small.tile([64, 512], F32, tag="bsb", name="b_sb")
                            nc.vector.reciprocal_approx_fast(out=b_sb[:], in_=b_ps[:])
                            nc.vector.tensor_mul(
                                out=ctxT[hp][po:po + 64, qbase:qbase + 512],
                                in0=ctx_ps[0:64, :], in1=b_sb[:])
                        return emit

                    for qc in range(QC):
                        qbase = qc * 512
                        kcs = [4 * qc] + list(range(0, 4 * qc)) + [4 * qc + 1, 4 * qc + 2, 4 * qc + 3]
                        for hp in range(4):
                            ctxA = ctx_ps_pool.tile([HD + 1, 512], F32, tag="ctxA", name="ctxA")
                            ctxB = ctx_ps_pool.tile([HD + 1, 512], F32, tag="ctxB", name="ctxB")
                            pend = None
                            for i, kc in enumerate(kcs):
                                off = max(0, 128 * kc - qbase)
                                sA = sc_ps.tile([NP, 512], F32, tag="sc", name="sA")
                                sB = sc_ps.tile([NP, 512], F32, tag="sc", name="sB")
                                # concurrent in the PE array: row groups 0-63 / 64-127
                                nc.tensor.matmul(
                                    sA[:, off:512],
                                    kT[hp][0:64, kc * NP:(kc + 1) * NP],
                                    qT[hp][0:64, qbase + off:qbase + 512],
                                    start=True, stop=True)
                                nc.tensor.matmul(
                                    sB[:, off:512],
                                    kT[hp][64:128, kc * NP:(kc + 1) * NP],
                                    qT[hp][64:128, qbase + off:qbase + 512],
                                    start=True, stop=True)
                                eA = e_pool.tile([NP, 512], BF16, tag="esb", name="eA")
                                eB = e_pool.tile([NP, 512], BF16, tag="esb", name="eB")
                                nc.scalar.activation(out=eA[:, off:512], in_=sA[:, off:512], func=AF.Exp)
                                nc.scalar.activation(out=eB[:, off:512], in_=sB[:, off:512], func=AF.Exp)
                                if 4 * qc <= kc:
                                    nc.vector.tensor_mul(
                                        out=eA[:, off:off + 128], in0=eA[:, off:off + 128], in1=mask_sb[:])
                                    nc.vector.tensor_mul(
                                        out=eB[:, off:off + 128], in0=eB[:, off:off + 128], in1=mask_sb[:])
                                if i == 0:
                                    # previous pair's softmax normalization, deferred
                                    # past this pair's first scores/exps so the
                                    # scalar-engine exp stream never stalls at the
                                    # pair boundary (must drain before this pair's
                                    # first ctx matmul reuses the ctx banks)
                                    flush_norm()
                                elif pending_proj:
                                    pending_proj.pop(0)()
                                if pend is not None:
                                    off_, eA_, eB_, kc_, first_ = pend
                                    nc.tensor.matmul(
                                        ctxA[:, off_:512], vaug[kc_][:, 2 * hp, :],
                                        eA_[:, off_:512], start=first_, stop=False)
                                    nc.tensor.matmul(
                                        ctxB[:, off_:512], vaug[kc_][:, 2 * hp + 1, :],
                                        eB_[:, off_:512], start=first_, stop=False)
                                pend = (off, eA, eB, kc, i == 0)
                            off_, eA_, eB_, kc_, first_ = pend
                            nc.tensor.matmul(
                                ctxA[:, off_:512], vaug[kc_][:, 2 * hp, :],
                                eA_[:, off_:512], start=first_, stop=True)
                            nc.tensor.matmul(
                                ctxB[:, off_:512], vaug[kc_][:, 2 * hp + 1, :],
                                eB_[:, off_:512], start=first_, stop=True)
                            pending_norm.append(make_norm(ctxA, hp, 0, qbase))
                            pending_norm.append(make_norm(ctxB, hp, 64, qbase))

                        if qc == 1:
                            pending_proj.extend(
                                (lambda qs=qs: emit_one_proj(qs, cc_in0, qs))
                                for qs in range(8))
                        if qc == 2:
                            while pending_proj:
                                pending_proj.pop(0)()
                            nc.gpsimd.collective_compute(
                                "ReduceScatter", OP.add,
                                replica_groups=[[0, 1], [2, 3], [4, 5], [6, 7]],
                                ins=[cc_in0[:].opt()], outs=[cc_out0[:].opt()])
                        if qc == 3:
                            flush_norm()
                            emit_proj(8, 16, cc_in1)

            # RS#2 issued OUTSIDE the attention pool scopes: the pool-stack
            # close drains all engines, so a collective issued inside would
            # serialize the whole FFN behind its completion.
            nc.gpsimd.collective_compute(
                "ReduceScatter", OP.add,
                replica_groups=[[0, 1], [2, 3], [4, 5], [6, 7]],
                ins=[cc_in1[:].opt()], outs=[cc_out1[:].opt()])

            # ---------------- FFN phase (1024 rows per core) ----------------
            with contextlib.ExitStack() as ffn:
                y_pool = ffn.enter_context(tc.tile_pool(name="y", bufs=1))
                y2T_pool = ffn.enter_context(tc.tile_pool(name="y2T", bufs=1))
                g_pool = ffn.enter_context(tc.tile_pool(name="g", bufs=64))
                yw = ffn.enter_context(tc.tile_pool(name="yw", bufs=3))
                lnst2 = ffn.enter_context(tc.tile_pool(name="lnst2", bufs=4))
                w1_pool = ffn.enter_context(tc.tile_pool(name="w1", bufs=5))
                w2_pool = ffn.enter_context(tc.tile_pool(name="w2", bufs=5))
                out_pool = ffn.enter_context(tc.tile_pool(name="outp", bufs=3))
                ff_ps_pool = ffn.enter_context(tc.tile_pool(name="ff_ps", bufs=2, space="PSUM"))
                z_ps_pool = ffn.enter_context(tc.tile_pool(name="z_ps", bufs=1, space="PSUM"))

                y = [y_pool.tile([NP, D], F32, tag=f"y{rc}", name=f"y{rc}") for rc in range(8)]
                y2T = y2T_pool.tile([NP, DC, RROWS], BF16)

                cc_outs = (cc_out0, cc_out1)
                for half in range(2):
                    # residual + LN2 for this half only, so half 0's FFN can
                    # run while the second ReduceScatter is still in flight
                    for r4 in range(4):
                        rc = half * 4 + r4
                        rs_sb = yw.tile([NP, D], BF16, tag="rs")
                        nc.sync.dma_start(out=rs_sb[:], in_=cc_outs[half][r4 * NP:(r4 + 1) * NP, :])
                        xr_sb = yw.tile([NP, D], F32, tag="xr")
                        nc.sync.dma_start(out=xr_sb[:], in_=xres_p[rc * NP:(rc + 1) * NP, :])
                        nc.vector.tensor_copy(out=y[rc][:], in_=rs_sb[:])
                        nc.vector.tensor_add(out=y[rc][:], in0=y[rc][:], in1=xr_sb[:])
                        nc.vector.tensor_add(out=y[rc][:], in0=y[rc][:], in1=bproj_bc[:])
                        mv = _ln_stats(nc, lnst2, y[rc][:], eps_sb)
                        y2_sb = yw.tile([NP, D], BF16, tag="y2")
                        nc.vector.tensor_scalar(
                            out=y2_sb[:], in0=y[rc][:],
                            scalar1=mv[:, 0:1], scalar2=mv[:, 1:2],
                            op0=OP.subtract, op1=OP.mult)
                        if apply_ln2_affine:
                            nc.vector.tensor_mul(out=y2_sb[:], in0=y2_sb[:], in1=ln2w_bc[:])
                            nc.vector.tensor_add(out=y2_sb[:], in0=y2_sb[:], in1=ln2b_bc[:])
                        for dc in range(DC):
                            t_ps = tp_ps.tile([NP, NP], BF16, tag="tp")
                            nc.tensor.transpose(t_ps[:], y2_sb[:, dc * NP:(dc + 1) * NP], ident_sb[:])
                            nc.scalar.copy(out=y2T[:, dc, rc * NP:(rc + 1) * NP], in_=t_ps[:])

                    hcols = slice(half * 512, (half + 1) * 512)
                    gts = []
                    for ffc in range(FC):
                        w1_sb = w1_pool.tile([NP, DC * NP], BF16, tag="w1")
                        nc.sync.dma_start(out=w1_sb[:], in_=wff1_p[ffc, :, :])
                        ff_ps = ff_ps_pool.tile([NP, 512], F32, tag="ffps")
                        for dc in range(DC):
                            nc.tensor.matmul(
                                ff_ps[:],
                                w1_sb[:, dc * NP:(dc + 1) * NP],
                                y2T[:, dc, hcols],
                                start=(dc == 0), stop=(dc == DC - 1))
                        g_sb = g_pool.tile([NP, 512], BF16, tag="g")
                        nc.scalar.activation(out=g_sb[:], in_=ff_ps[:], func=AF.Gelu,
                                             bias=bff1_sb[:, ffc:ffc + 1], scale=1.0)
                        gts.append(g_sb)
                    for nch in range(2):
                        ncols = slice(nch * 512, (nch + 1) * 512)
                        z_pss = [z_ps_pool.tile([NP, 512], F32, tag=f"zps{r}", name=f"zps{r}") for r in range(4)]
                        for ffc in range(FC):
                            w2_sb = w2_pool.tile([NP, 512], BF16, tag="w2")
                            nc.sync.dma_start(out=w2_sb[:], in_=wff2_p[ffc, :, ncols])
                            for r4 in range(4):
                                nc.tensor.matmul(
                                    z_pss[r4][:],
                                    gts[ffc][:, r4 * NP:(r4 + 1) * NP],
                                    w2_sb[:],
                                    start=(ffc == 0), stop=(ffc == FC - 1))
                        for r4 in range(4):
                            rc = half * 4 + r4
                            o_sb = out_pool.tile([NP, 512], F32, tag="osb")
                            nc.vector.tensor_add(out=o_sb[:], in0=z_pss[r4][:], in1=y[rc][:, ncols])
                            nc.vector.tensor_add(out=o_sb[:], in0=o_sb[:], in1=bff2_bc[:, ncols])
                            nc.sync.dma_start(out=out_p[rc * NP:(rc + 1) * NP, ncols], in_=o_sb[:])

    nc.compile()
    return nc


# ------------------------- host-side driver -------------------------

_BF = ml_dtypes.bfloat16


def _core_rows(g):
    return np.r_[512 * g:512 * g + 512, 1024 + 512 * g:1536 + 512 * g]


def _prep_core_inputs(inputs, b, g):
    x = np.asarray(inputs["x"], np.float32)
    w_qkv = np.asarray(inputs["w_qkv"], np.float32).reshape(D, H, HD, 3)
    hs = slice(g * GH, (g + 1) * GH)
    w_k = w_qkv[:, hs, :, 0].reshape(D, GH * HD)
    w_q = (w_qkv[:, hs, :, 1] * (HD ** -0.5)).reshape(D, GH * HD)
    w_v = w_qkv[:, hs, :, 2].reshape(D, GH * HD)

    def tile_kxm(w):  # [D, 512] -> [128, DC, 512]
        return np.ascontiguousarray(w.reshape(DC, NP, GH * HD).transpose(1, 0, 2))

    w_proj = np.asarray(inputs["w_proj"], np.float32)
    wp = np.ascontiguousarray(
        w_proj[g * 512:(g + 1) * 512, :].reshape(4, NP, D).transpose(1, 0, 2))

    w_ff1 = np.asarray(inputs["w_ff1"], np.float32)
    w1t = np.ascontiguousarray(
        w_ff1.reshape(DC, NP, FC, NP).transpose(2, 1, 0, 3).reshape(FC, NP, DC * NP))
    w_ff2 = np.asarray(inputs["w_ff2"], np.float32)
    w2t = np.ascontiguousarray(w_ff2.reshape(FC, NP, D))

    j = np.arange(NP)[:, None]
    i = np.arange(NP)[None, :]
    mask = (j <= i).astype(np.float32)

    return {
        "x": np.ascontiguousarray(x[b]),
        "x_res": np.ascontiguousarray(x[b][_core_rows(g)]),
        "wq": tile_kxm(w_q).astype(_BF),
        "wk": tile_kxm(w_k).astype(_BF),
        "wv": tile_kxm(w_v).astype(_BF),
        "wproj": wp.astype(_BF),
        "wff1": w1t.astype(_BF),
        "wff2": w2t.astype(_BF),
        "bproj": np.asarray(inputs["b_proj"], np.float32).reshape(1, D).copy(),
        "bff1": np.ascontiguousarray(np.asarray(inputs["b_ff1"], np.float32).reshape(FC, NP).T),
        "bff2": np.asarray(inputs["b_ff2"], np.float32).reshape(1, D).copy(),
        "ident": np.eye(NP, dtype=_BF),
        "mask": mask.astype(_BF),
    }


_NC_CACHE = {}


def kernel(**inputs):
    ln1w = np.asarray(inputs["ln1_w"], np.float32)
    ln1b = np.asarray(inputs["ln1_b"], np.float32)
    ln2w = np.asarray(inputs["ln2_w"], np.float32)
    ln2b = np.asarray(inputs["ln2_b"], np.float32)
    a1 = not (np.allclose(ln1w, 1.0) and np.allclose(ln1b, 0.0))
    a2 = not (np.allclose(ln2w, 1.0) and np.allclose(ln2b, 0.0))

    key = (a1, a2)
    if key not in _NC_CACHE:
        _NC_CACHE[key] = build(apply_ln1_affine=a1, apply_ln2_affine=a2)
    nc = _NC_CACHE[key]

    in_maps = []
    for core in range(8):
        b, g = core // 2, core % 2
        m = _prep_core_inputs(inputs, b, g)
        if a1:
            m["ln1w"] = ln1w.reshape(1, D).copy()
            m["ln1b"] = ln1b.reshape(1, D).copy()
        if a2:
            m["ln2w"] = ln2w.reshape(1, D).copy()
            m["ln2b"] = ln2b.reshape(1, D).copy()
        in_maps.append(m)

    res = run_bass_kernel_spmd(nc, in_maps, core_ids=list(range(8)))

    out = np.empty((B, S, D), np.float32)
    for core in range(8):
        b, g = core // 2, core % 2
        out[b][_core_rows(g)] = res.results[core]["out"]
    return out


# revision 24
# speedup vs baseline: 1.0421x; 1.0041x over previous
"""Trainium2 Bass kernel for a pre-LN transformer block (attention + FFN).

Sharding over 8 NeuronCores: core (b, g) = batch b (0..3) x head-group g (0..1).
Each core runs LN1 + QKV (its 8 heads) + causal attention + its slice of the
output projection for its batch; a pairwise bf16 ReduceScatter (2 chunks,
first overlapped with attention of the second query half) sums the two
head-groups' partial attn_out; each core then finishes 1024 rows
(residual + LN2 + full FFN).

Attention inner loop: the two heads of a q/k pair live on partitions 0:64 and
64:128, so their K=64 score matmuls run CONCURRENTLY in the PE array via
row-group tiling (tile_position (0,0)/(64,0)); a 1-step software pipeline
issues scores(i)/exp(i) before ctx(i-1) so the PE never stalls on the scalar
engine and the HAM clock stays warm. Softmax denominators via a ones-column
on V and reciprocal_approx_fast.
"""
import sys

if "/opt/trn_rl_repo" not in sys.path:
    sys.path.insert(0, "/opt/trn_rl_repo")

import contextlib

import numpy as np
import ml_dtypes

import concourse.bass as bass
import concourse.bacc as bacc
import concourse.tile as tile
from concourse import mybir
from concourse.bass_utils import run_bass_kernel_spmd

F32 = mybir.dt.float32
F32R = mybir.dt.float32r
BF16 = mybir.dt.bfloat16
AF = mybir.ActivationFunctionType
OP = mybir.AluOpType

B, S, D, H = 4, 2048, 1024, 16
HD = D // H
FF = 4 * D
EPS = 1e-5
GH = 8          # heads per core
NP = 128        # partitions
SC = S // NP    # 16 seq chunks of 128
DC = D // NP    # 8 d-chunks
QC = S // 512   # 4 q-chunks of 512
FC = FF // NP   # 32 ff chunks of 128
RROWS = S // 2  # 1024 rows finalized per core


def _ln_stats(nc, pool, x_ap, eps_sb):
    """bn_stats/bn_aggr mean+rstd for a [128, D] fp32 tile. Returns mv tile;
    mv[:,0:1]=mean, mv[:,1:2]=rstd (after rsqrt)."""
    stats = pool.tile([NP, 2, 6], F32, tag="lnstats")
    nc.vector.bn_stats(out=stats[:, 0, :], in_=x_ap[:, 0:512])
    nc.vector.bn_stats(out=stats[:, 1, :], in_=x_ap[:, 512:1024])
    mv = pool.tile([NP, 2], F32, tag="lnmv")
    nc.vector.bn_aggr(out=mv[:], in_=stats[:])
    nc.scalar.activation(out=mv[:, 1:2], in_=mv[:, 1:2], func=AF.Sqrt,
                         bias=eps_sb[:], scale=1.0)
    nc.vector.reciprocal(out=mv[:, 1:2], in_=mv[:, 1:2])
    return mv


def build(apply_ln1_affine=False, apply_ln2_affine=False):
    nc = bacc.Bacc("TRN2", num_devices=8)

    # ---- DRAM parameters (per-core shards, laid out host-side) ----
    x_p = nc.declare_dram_parameter("x", [S, D], F32, isOutput=False)
    xres_p = nc.declare_dram_parameter("x_res", [RROWS, D], F32, isOutput=False)
    wq_p = nc.declare_dram_parameter("wq", [NP, DC, 512], BF16, isOutput=False)
    wk_p = nc.declare_dram_parameter("wk", [NP, DC, 512], BF16, isOutput=False)
    wv_p = nc.declare_dram_parameter("wv", [NP, DC, 512], BF16, isOutput=False)
    wproj_p = nc.declare_dram_parameter("wproj", [NP, 4, D], BF16, isOutput=False)
    wff1_p = nc.declare_dram_parameter("wff1", [FC, NP, DC * NP], BF16, isOutput=False)
    wff2_p = nc.declare_dram_parameter("wff2", [FC, NP, D], BF16, isOutput=False)
    bproj_p = nc.declare_dram_parameter("bproj", [1, D], F32, isOutput=False)
    bff1_p = nc.declare_dram_parameter("bff1", [NP, FC], F32, isOutput=False)
    bff2_p = nc.declare_dram_parameter("bff2", [1, D], F32, isOutput=False)
    ident_p = nc.declare_dram_parameter("ident", [NP, NP], BF16, isOutput=False)
    mask_p = nc.declare_dram_parameter("mask", [NP, NP], BF16, isOutput=False)
    if apply_ln1_affine:
        ln1w_p = nc.declare_dram_parameter("ln1w", [1, D], F32, isOutput=False)
        ln1b_p = nc.declare_dram_parameter("ln1b", [1, D], F32, isOutput=False)
    if apply_ln2_affine:
        ln2w_p = nc.declare_dram_parameter("ln2w", [1, D], F32, isOutput=False)
        ln2b_p = nc.declare_dram_parameter("ln2b", [1, D], F32, isOutput=False)
    out_p = nc.declare_dram_parameter("out", [RROWS, D], F32, isOutput=True)

    with tile.TileContext(nc) as tc:
        with contextlib.ExitStack() as stack:
            # ---------------- persistent pools ----------------
            const = stack.enter_context(tc.tile_pool(name="const", bufs=1))
            tp_ps = stack.enter_context(tc.tile_pool(name="tp_ps", bufs=2, space="PSUM"))
            dram = stack.enter_context(tc.tile_pool(name="dram", bufs=1, space="DRAM"))
            ctxT_pool = stack.enter_context(tc.tile_pool(name="ctxT", bufs=1))
            wp_pool = stack.enter_context(tc.tile_pool(name="wp", bufs=1))
            small = stack.enter_context(tc.tile_pool(name="small", bufs=2))

            eps_sb = const.tile([NP, 1], F32)
            nc.vector.memset(eps_sb[:], EPS)
            ones_f32 = const.tile([NP, 64], F32)
            nc.vector.memset(ones_f32[:], 1.0)
            ones_sb = const.tile([NP, 64], F32R)
            nc.vector.tensor_copy(out=ones_sb[:], in_=ones_f32[:])
            ident_sb = const.tile([NP, NP], BF16)
            nc.sync.dma_start(out=ident_sb[:], in_=ident_p[:, :])
            mask_sb = const.tile([NP, NP], BF16)
            nc.sync.dma_start(out=mask_sb[:], in_=mask_p[:, :])
            bproj_bc = const.tile([NP, D], F32)
            nc.gpsimd.dma_start(out=bproj_bc[:], in_=bproj_p[:, :].to_broadcast((NP, D)))
            bff2_bc = const.tile([NP, D], F32)
            nc.gpsimd.dma_start(out=bff2_bc[:], in_=bff2_p[:, :].to_broadcast((NP, D)))
            bff1_sb = const.tile([NP, FC], F32)
            nc.sync.dma_start(out=bff1_sb[:], in_=bff1_p[:, :])
            if apply_ln1_affine:
                ln1w_bc = const.tile([NP, D], F32)
                nc.gpsimd.dma_start(out=ln1w_bc[:], in_=ln1w_p[:, :].to_broadcast((NP, D)))
                ln1b_bc = const.tile([NP, D], F32)
                nc.gpsimd.dma_start(out=ln1b_bc[:], in_=ln1b_p[:, :].to_broadcast((NP, D)))
            if apply_ln2_affine:
                ln2w_bc = const.tile([NP, D], F32)
                nc.gpsimd.dma_start(out=ln2w_bc[:], in_=ln2w_p[:, :].to_broadcast((NP, D)))
                ln2b_bc = const.tile([NP, D], F32)
                nc.gpsimd.dma_start(out=ln2b_bc[:], in_=ln2b_p[:, :].to_broadcast((NP, D)))

            # bf16 collective buffers, 2 chunks of 1024 rows each
            cc_in0 = dram.tile([RROWS, D], BF16)
            cc_in1 = dram.tile([RROWS, D], BF16)
            cc_out0 = dram.tile([RROWS // 2, D], BF16)
            cc_out1 = dram.tile([RROWS // 2, D], BF16)

            ctxT = [ctxT_pool.tile([NP, S], BF16, tag=f"ctxT{p}", name=f"ctxT{p}") for p in range(4)]
            wp_sb = wp_pool.tile([NP, 4, D], BF16)

            # ---------------- attention-lifetime pools ----------------
            with contextlib.ExitStack() as att:
                qkT_pool = att.enter_context(tc.tile_pool(name="qkT", bufs=1))
                vaug_pool = att.enter_context(tc.tile_pool(name="vaug", bufs=1))
                e_pool = att.enter_context(tc.tile_pool(name="e", bufs=6))
                attn_pool = att.enter_context(tc.tile_pool(name="attn", bufs=2))
                sc_ps = att.enter_context(tc.tile_pool(name="sc_ps", bufs=4, space="PSUM"))

                vaug = [vaug_pool.tile([NP, GH, HD + 1], BF16, tag=f"v{sc}", name=f"vaug{sc}") for sc in range(SC)]
                for sc in range(SC):
                    nc.vector.memset(vaug[sc][:, :, HD:HD + 1], 1.0)
                qT = [qkT_pool.tile([NP, S], BF16, tag=f"qT{p}", name=f"qT{p}") for p in range(4)]
                kT = [qkT_pool.tile([NP, S], BF16, tag=f"kT{p}", name=f"kT{p}") for p in range(4)]

                # ---- prep phase: LN1 + hT, v rows, q/k projections ----
                with contextlib.ExitStack() as prep:
                    wqkv = prep.enter_context(tc.tile_pool(name="wqkv", bufs=1))
                    hT_pool = prep.enter_context(tc.tile_pool(name="hT", bufs=1))
                    xln = prep.enter_context(tc.tile_pool(name="xln", bufs=2))
                    hrow = prep.enter_context(tc.tile_pool(name="hrow", bufs=3))
                    lnst = prep.enter_context(tc.tile_pool(name="lnst", bufs=4))
                    qkv_ps = prep.enter_context(tc.tile_pool(name="qkv_ps", bufs=2, space="PSUM"))

                    wq_sb = wqkv.tile([NP, DC, 512], BF16)
                    wk_sb = wqkv.tile([NP, DC, 512], BF16)
                    wv_sb = wqkv.tile([NP, DC, 512], BF16)
                    nc.sync.dma_start(out=wq_sb[:], in_=wq_p[:, :, :])
                    nc.sync.dma_start(out=wk_sb[:], in_=wk_p[:, :, :])
                    nc.sync.dma_start(out=wv_sb[:], in_=wv_p[:, :, :])

                    hT = hT_pool.tile([NP, DC, S], BF16)        # LN1(x)^T

                    # LN1 + transpose h -> hT (copies on the scalar engine;
                    # the vector engine is the prep-phase bottleneck)
                    for sc in range(SC):
                        x_sb = xln.tile([NP, D], F32, tag="x")
                        nc.sync.dma_start(out=x_sb[:], in_=x_p[sc * NP:(sc + 1) * NP, :])
                        mv = _ln_stats(nc, lnst, x_sb[:], eps_sb)
                        h_sb = hrow.tile([NP, D], BF16, tag="h")
                        nc.vector.tensor_scalar(
                            out=h_sb[:], in0=x_sb[:],
                            scalar1=mv[:, 0:1], scalar2=mv[:, 1:2],
                            op0=OP.subtract, op1=OP.mult)
                        if apply_ln1_affine:
                            nc.vector.tensor_mul(out=h_sb[:], in0=h_sb[:], in1=ln1w_bc[:])
                            nc.vector.tensor_add(out=h_sb[:], in0=h_sb[:], in1=ln1b_bc[:])
                        for dc in range(DC):
                            t_ps = tp_ps.tile([NP, NP], BF16, tag="tp")
                            nc.tensor.transpose(t_ps[:], h_sb[:, dc * NP:(dc + 1) * NP], ident_sb[:])
                            dst = hT[:, dc, sc * NP:(sc + 1) * NP]
                            if dc % 2 == 0:
                                nc.scalar.copy(out=dst, in_=t_ps[:])
                            else:
                                nc.vector.tensor_copy(out=dst, in_=t_ps[:])

                    # v rows: per seq chunk, out [128 seq, 512 all heads]
                    for sc in range(SC):
                        ps = qkv_ps.tile([NP, 512], F32, tag="qkvps")
                        for dc in range(DC):
                            nc.tensor.matmul(
                                ps[:],
                                hT[:, dc, sc * NP:(sc + 1) * NP],
                                wv_sb[:, dc, :],
                                start=(dc == 0), stop=(dc == DC - 1))
                        nc.vector.tensor_copy(
                            out=vaug[sc][:, :, 0:HD],
                            in_=ps[:].rearrange("p (h d) -> p h d", h=GH))

                    # q/k projections for ALL pairs
                    for hp in range(4):
                        for s4 in range(QC):
                            cols = slice(s4 * 512, (s4 + 1) * 512)
                            for w_sb, dst in ((wq_sb, qT[hp]), (wk_sb, kT[hp])):
                                ps = qkv_ps.tile([NP, 512], F32, tag="qkvps")
                                for dc in range(DC):
                                    nc.tensor.matmul(
                                        ps[:],
                                        w_sb[:, dc, hp * NP:(hp + 1) * NP],
                                        hT[:, dc, cols],
                                        start=(dc == 0), stop=(dc == DC - 1))
                                nc.vector.tensor_copy(out=dst[:, cols], in_=ps[:])

                # wproj load deferred out of the startup DMA window (first
                # needed by emit_proj at qc==1, ~100us later)
                nc.sync.dma_start(out=wp_sb[:], in_=wproj_p[:, :, :])

                # ---- attention: qc-major, both heads of a pair in flight ----
                with contextlib.ExitStack() as att2:
                    ctx_ps_pool = att2.enter_context(tc.tile_pool(name="ctx_ps", bufs=1, space="PSUM"))

                    def emit_one_proj(qs, cc_dst, r):
                        attn_sb = attn_pool.tile([NP, D], BF16, tag="attnsb", name="attn_sb")
                        for nch in range(2):
                            a_ps = sc_ps.tile([NP, 512], F32, tag="sc", name="a_ps")
                            for pair in range(4):
                                nc.tensor.matmul(
                                    a_ps[:],
                                    ctxT[pair][:, qs * NP:(qs + 1) * NP],
                                    wp_sb[:, pair, nch * 512:(nch + 1) * 512],
                                    start=(pair == 0), stop=(pair == 3))
                            nc.vector.tensor_copy(out=attn_sb[:, nch * 512:(nch + 1) * 512], in_=a_ps[:])
                        nc.sync.dma_start(out=cc_dst[r * NP:(r + 1) * NP, :], in_=attn_sb[:])

                    def emit_proj(qs_lo, qs_hi, cc_dst):
                        for qs in range(qs_lo, qs_hi):
                            emit_one_proj(qs, cc_dst, qs - qs_lo)

                    # proj chains for rows 0-1023, interleaved one-per-step
                    # into qc==2's attention so the scalar exp stream is
                    # never starved by a block of PE-only proj work
                    pending_proj = []

                    pending_norm = []

                    def flush_norm():
                        while pending_norm:
                            pending_norm.pop(0)()

                    def make_norm(ctx_ps, hp, po, qbase):
                        def emit():
                            den = small.tile([NP, 512], F32R, tag="den", name="den")
                            nc.vector.tensor_copy(out=den[64:65, :], in_=ctx_ps[64:65, :])
                            b_ps = sc_ps.tile([64, 512], F32, tag="sc", name="b_ps")
                            nc.tensor.matmul(b_ps[:], ones_sb[64:65, :], den[64:65, :],
                                             start=True, stop=True)
                            b_sb = small.tile([64, 512], F32, tag="bsb", name="b_sb")
                            nc.vector.reciprocal_approx_fast(out=b_sb[:], in_=b_ps[:])
                            nc.vector.tensor_mul(
                                out=ctxT[hp][po:po + 64, qbase:qbase + 512],
                                in0=ctx_ps[0:64, :], in1=b_sb[:])
                        return emit

                    for qc in range(QC):
                        qbase = qc * 512
                        kcs = [4 * qc] + list(range(0, 4 * qc)) + [4 * qc + 1, 4 * qc + 2, 4 * qc + 3]
                        for hp in range(4):
                            ctxA = ctx_ps_pool.tile([HD + 1, 512], F32, tag="ctxA", name="ctxA")
                            ctxB = ctx_ps_pool.tile([HD + 1, 512], F32, tag="ctxB", name="ctxB")
                            pend = None
                            for i, kc in enumerate(kcs):
                                off = max(0, 128 * kc - qbase)
                                sA = sc_ps.tile([NP, 512], F32, tag="sc", name="sA")
                                sB = sc_ps.tile([NP, 512], F32, tag="sc", name="sB")
                                # concurrent in the PE array: row groups 0-63 / 64-127
                                nc.tensor.matmul(
                                    sA[:, off:512],
                                    kT[hp][0:64, kc * NP:(kc + 1) * NP],
                                    qT[hp][0:64, qbase + off:qbase + 512],
                                    start=True, stop=True)
                                nc.tensor.matmul(
                                    sB[:, off:512],
                                    kT[hp][64:128, kc * NP:(kc + 1) * NP],
                                    qT[hp][64:128, qbase + off:qbase + 512],
                                    start=True, stop=True)
                                eA = e_pool.tile([NP, 512], BF16, tag="esb", name="eA")
                                eB = e_pool.tile([NP, 512], BF16, tag="esb", name="eB")
                                nc.scalar.activation(out=eA[:, off:512], in_=sA[:, off:512], func=AF.Exp)
                                nc.scalar.activation(out=eB[:, off:512], in_=sB[:, off:512], func=AF.Exp)
                                if 4 * qc <= kc:
                                    nc.vector.tensor_mul(
                                        out=eA[:, off:off + 128], in0=eA[:, off:off + 128], in1=mask_sb[:])
                                    nc.vector.tensor_mul(
                                        out=eB[:, off:off + 128], in0=eB[:, off:off + 128], in1=mask_sb[:])
                                if i == 0:
                                    # previous pair's softmax normalization, deferred
                                    # past this pair's first scores/exps so the
                                    # scalar-engine exp stream never stalls at the
                                    # pair boundary (must drain before this pair's
                                    # first ctx matmul reuses the ctx banks)
                                    flush_norm()
                                elif pending_proj:
                                    pending_proj.pop(0)()
                                if pend is not None:
                                    off_, eA_, eB_, kc_, first_ = pend
                                    nc.tensor.matmul(
                                        ctxA[:, off_:512], vaug[kc_][:, 2 * hp, :],
                                        eA_[:, off_:512], start=first_, stop=False)
                                    nc.tensor.matmul(
                                        ctxB[:, off_:512], vaug[kc_][:, 2 * hp + 1, :],
                                        eB_[:, off_:512], start=first_, stop=False)
                                pend = (off, eA, eB, kc, i == 0)
                            off_, eA_, eB_, kc_, first_ = pend
                            nc.tensor.matmul(
                                ctxA[:, off_:512], vaug[kc_][:, 2 * hp, :],
                                eA_[:, off_:512], start=first_, stop=True)
                            nc.tensor.matmul(
                                ctxB[:, off_:512], vaug[kc_][:, 2 * hp + 1, :],
                                eB_[:, off_:512], start=first_, stop=True)
                            pending_norm.append(make_norm(ctxA, hp, 0, qbase))
                            pending_norm.append(make_norm(ctxB, hp, 64, qbase))

                        if qc == 1:
                            pending_proj.extend(
                                (lambda qs=qs: emit_one_proj(qs, cc_in0, qs))
                                for qs in range(8))
                        if qc == 2:
                            while pending_proj:
                                pending_proj.pop(0)()
                            nc.gpsimd.collective_compute(
                                "ReduceScatter", OP.add,
                                replica_groups=[[0, 1], [2, 3], [4, 5], [6, 7]],
                                ins=[cc_in0[:].opt()], outs=[cc_out0[:].opt()])
                            # qs 8-11 (rows 1024-1535) only need qc<=2 context;
                            # interleave them into qc==3's attention steps
                            pending_proj.extend(
                                (lambda qs=qs: emit_one_proj(qs, cc_in1, qs - 8))
                                for qs in range(8, 12))
                        if qc == 3:
                            while pending_proj:
                                pending_proj.pop(0)()
                            flush_norm()
                            for qs in range(12, 16):
                                emit_one_proj(qs, cc_in1, qs - 8)

            # RS#2 issued OUTSIDE the attention pool scopes: the pool-stack
            # close drains all engines, so a collective issued inside would
            # serialize the whole FFN behind its completion.
            nc.gpsimd.collective_compute(
                "ReduceScatter", OP.add,
                replica_groups=[[0, 1], [2, 3], [4, 5], [6, 7]],
                ins=[cc_in1[:].opt()], outs=[cc_out1[:].opt()])

            # ---------------- FFN phase (1024 rows per core) ----------------
            with contextlib.ExitStack() as ffn:
                y_pool = ffn.enter_context(tc.tile_pool(name="y", bufs=1))
                y2T_pool = ffn.enter_context(tc.tile_pool(name="y2T", bufs=1))
                g_pool = ffn.enter_context(tc.tile_pool(name="g", bufs=64))
                yw = ffn.enter_context(tc.tile_pool(name="yw", bufs=3))
                lnst2 = ffn.enter_context(tc.tile_pool(name="lnst2", bufs=4))
                w1_pool = ffn.enter_context(tc.tile_pool(name="w1", bufs=5))
                w2_pool = ffn.enter_context(tc.tile_pool(name="w2", bufs=5))
                out_pool = ffn.enter_context(tc.tile_pool(name="outp", bufs=3))
                ff_ps_pool = ffn.enter_context(tc.tile_pool(name="ff_ps", bufs=2, space="PSUM"))
                z_ps_pool = ffn.enter_context(tc.tile_pool(name="z_ps", bufs=1, space="PSUM"))

                y = [y_pool.tile([NP, D], F32, tag=f"y{rc}", name=f"y{rc}") for rc in range(8)]
                y2T = y2T_pool.tile([NP, DC, RROWS], BF16)

                cc_outs = (cc_out0, cc_out1)
                for half in range(2):
                    # residual + LN2 for this half only, so half 0's FFN can
                    # run while the second ReduceScatter is still in flight
                    for r4 in range(4):
                        rc = half * 4 + r4
                        rs_sb = yw.tile([NP, D], BF16, tag="rs")
                        nc.sync.dma_start(out=rs_sb[:], in_=cc_outs[half][r4 * NP:(r4 + 1) * NP, :])
                        xr_sb = yw.tile([NP, D], F32, tag="xr")
                        nc.sync.dma_start(out=xr_sb[:], in_=xres_p[rc * NP:(rc + 1) * NP, :])
                        nc.vector.tensor_copy(out=y[rc][:], in_=rs_sb[:])
                        nc.vector.tensor_add(out=y[rc][:], in0=y[rc][:], in1=xr_sb[:])
                        nc.vector.tensor_add(out=y[rc][:], in0=y[rc][:], in1=bproj_bc[:])
                        mv = _ln_stats(nc, lnst2, y[rc][:], eps_sb)
                        y2_sb = yw.tile([NP, D], BF16, tag="y2")
                        nc.vector.tensor_scalar(
                            out=y2_sb[:], in0=y[rc][:],
                            scalar1=mv[:, 0:1], scalar2=mv[:, 1:2],
                            op0=OP.subtract, op1=OP.mult)
                        if apply_ln2_affine:
                            nc.vector.tensor_mul(out=y2_sb[:], in0=y2_sb[:], in1=ln2w_bc[:])
                            nc.vector.tensor_add(out=y2_sb[:], in0=y2_sb[:], in1=ln2b_bc[:])
                        for dc in range(DC):
                            t_ps = tp_ps.tile([NP, NP], BF16, tag="tp")
                            nc.tensor.transpose(t_ps[:], y2_sb[:, dc * NP:(dc + 1) * NP], ident_sb[:])
                            nc.scalar.copy(out=y2T[:, dc, rc * NP:(rc + 1) * NP], in_=t_ps[:])

                    hcols = slice(half * 512, (half + 1) * 512)
                    gts = []
                    for ffc in range(FC):
                        w1_sb = w1_pool.tile([NP, DC * NP], BF16, tag="w1")
                        nc.sync.dma_start(out=w1_sb[:], in_=wff1_p[ffc, :, :])
                        ff_ps = ff_ps_pool.tile([NP, 512], F32, tag="ffps")
                        for dc in range(DC):
                            nc.tensor.matmul(
                                ff_ps[:],
                                w1_sb[:, dc * NP:(dc + 1) * NP],
                                y2T[:, dc, hcols],
                                start=(dc == 0), stop=(dc == DC - 1))
                        g_sb = g_pool.tile([NP, 512], BF16, tag="g")
                        nc.scalar.activation(out=g_sb[:], in_=ff_ps[:], func=AF.Gelu,
                                             bias=bff1_sb[:, ffc:ffc + 1], scale=1.0)
                        gts.append(g_sb)
                    for nch in range(2):
                        ncols = slice(nch * 512, (nch + 1) * 512)
                        z_pss = [z_ps_pool.tile([NP, 512], F32, tag=f"zps{r}", name=f"zps{r}") for r in range(4)]
                        for ffc in range(FC):
                            w2_sb = w2_pool.tile([NP, 512], BF16, tag="w2")
                            nc.sync.dma_start(out=w2_sb[:], in_=wff2_p[ffc, :, ncols])
                            for r4 in range(4):
                                nc.tensor.matmul(
                                    z_pss[r4][:],
                                    gts[ffc][:, r4 * NP:(r4 + 1) * NP],
                                    w2_sb[:],
                                    start=(ffc == 0), stop=(ffc == FC - 1))
                        for r4 in range(4):
                            rc = half * 4 + r4
                            o_sb = out_pool.tile([NP, 512], F32, tag="osb")
                            nc.vector.tensor_add(out=o_sb[:], in0=z_pss[r4][:], in1=y[rc][:, ncols])
                            nc.vector.tensor_add(out=o_sb[:], in0=o_sb[:], in1=bff2_bc[:, ncols])
                            nc.sync.dma_start(out=out_p[rc * NP:(rc + 1) * NP, ncols], in_=o_sb[:])

    nc.compile()
    return nc


# ------------------------- host-side driver -------------------------

_BF = ml_dtypes.bfloat16


def _core_rows(g):
    return np.r_[512 * g:512 * g + 512, 1024 + 512 * g:1536 + 512 * g]


def _prep_core_inputs(inputs, b, g):
    x = np.asarray(inputs["x"], np.float32)
    w_qkv = np.asarray(inputs["w_qkv"], np.float32).reshape(D, H, HD, 3)
    hs = slice(g * GH, (g + 1) * GH)
    w_k = w_qkv[:, hs, :, 0].reshape(D, GH * HD)
    w_q = (w_qkv[:, hs, :, 1] * (HD ** -0.5)).reshape(D, GH * HD)
    w_v = w_qkv[:, hs, :, 2].reshape(D, GH * HD)

    def tile_kxm(w):  # [D, 512] -> [128, DC, 512]
        return np.ascontiguousarray(w.reshape(DC, NP, GH * HD).transpose(1, 0, 2))

    w_proj = np.asarray(inputs["w_proj"], np.float32)
    wp = np.ascontiguousarray(
        w_proj[g * 512:(g + 1) * 512, :].reshape(4, NP, D).transpose(1, 0, 2))

    w_ff1 = np.asarray(inputs["w_ff1"], np.float32)
    w1t = np.ascontiguousarray(
        w_ff1.reshape(DC, NP, FC, NP).transpose(2, 1, 0, 3).reshape(FC, NP, DC * NP))
    w_ff2 = np.asarray(inputs["w_ff2"], np.float32)
    w2t = np.ascontiguousarray(w_ff2.reshape(FC, NP, D))

    j = np.arange(NP)[:, None]
    i = np.arange(NP)[None, :]
    mask = (j <= i).astype(np.float32)

    return {
        "x": np.ascontiguousarray(x[b]),
        "x_res": np.ascontiguousarray(x[b][_core_rows(g)]),
        "wq": tile_kxm(w_q).astype(_BF),
        "wk": tile_kxm(w_k).astype(_BF),
        "wv": tile_kxm(w_v).astype(_BF),
        "wproj": wp.astype(_BF),
        "wff1": w1t.astype(_BF),
        "wff2": w2t.astype(_BF),
        "bproj": np.asarray(inputs["b_proj"], np.float32).reshape(1, D).copy(),
        "bff1": np.ascontiguousarray(np.asarray(inputs["b_ff1"], np.float32).reshape(FC, NP).T),
        "bff2": np.asarray(inputs["b_ff2"], np.float32).reshape(1, D).copy(),
        "ident": np.eye(NP, dtype=_BF),
        "mask": mask.astype(_BF),
    }


_NC_CACHE = {}


def kernel(**inputs):
    ln1w = np.asarray(inputs["ln1_w"], np.float32)
    ln1b = np.asarray(inputs["ln1_b"], np.float32)
    ln2w = np.asarray(inputs["ln2_w"], np.float32)
    ln2b = np.asarray(inputs["ln2_b"], np.float32)
    a1 = not (np.allclose(ln1w, 1.0) and np.allclose(ln1b, 0.0))
    a2 = not (np.allclose(ln2w, 1.0) and np.allclose(ln2b, 0.0))

    key = (a1, a2)
    if key not in _NC_CACHE:
        _NC_CACHE[key] = build(apply_ln1_affine=a1, apply_ln2_affine=a2)
    nc = _NC_CACHE[key]

    in_maps = []
    for core in range(8):
        b, g = core // 2, core % 2
        m = _prep_core_inputs(inputs, b, g)
        if a1:
            m["ln1w"] = ln1w.reshape(1, D).copy()
            m["ln1b"] = ln1b.reshape(1, D).copy()
        if a2:
            m["ln2w"] = ln2w.reshape(1, D).copy()
            m["ln2b"] = ln2b.reshape(1, D).copy()
        in_maps.append(m)

    res = run_bass_kernel_spmd(nc, in_maps, core_ids=list(range(8)))

    out = np.empty((B, S, D), np.float32)
    for core in range(8):
        b, g = core // 2, core % 2
        out[b][_core_rows(g)] = res.results[core]["out"]
    return out


# revision 27
# speedup vs baseline: 1.0495x; 1.0071x over previous
"""Trainium2 Bass kernel for a pre-LN transformer block (attention + FFN).

Sharding over 8 NeuronCores: core (b, g) = batch b (0..3) x head-group g (0..1).
Each core runs LN1 + QKV (its 8 heads) + causal attention + its slice of the
output projection for its batch; a pairwise bf16 ReduceScatter (2 chunks,
first overlapped with attention of the second query half) sums the two
head-groups' partial attn_out; each core then finishes 1024 rows
(residual + LN2 + full FFN).

Attention inner loop: the two heads of a q/k pair live on partitions 0:64 and
64:128, so their K=64 score matmuls run CONCURRENTLY in the PE array via
row-group tiling (tile_position (0,0)/(64,0)); a 1-step software pipeline
issues scores(i)/exp(i) before ctx(i-1) so the PE never stalls on the scalar
engine and the HAM clock stays warm. Softmax denominators via a ones-column
on V and reciprocal_approx_fast.
"""
import sys

if "/opt/trn_rl_repo" not in sys.path:
    sys.path.insert(0, "/opt/trn_rl_repo")

import contextlib

import numpy as np
import ml_dtypes

import concourse.bass as bass
import concourse.bacc as bacc
import concourse.tile as tile
from concourse import mybir
from concourse.bass_utils import run_bass_kernel_spmd

F32 = mybir.dt.float32
F32R = mybir.dt.float32r
BF16 = mybir.dt.bfloat16
AF = mybir.ActivationFunctionType
OP = mybir.AluOpType

B, S, D, H = 4, 2048, 1024, 16
HD = D // H
FF = 4 * D
EPS = 1e-5
GH = 8          # heads per core
NP = 128        # partitions
SC = S // NP    # 16 seq chunks of 128
DC = D // NP    # 8 d-chunks
QC = S // 512   # 4 q-chunks of 512
FC = FF // NP   # 32 ff chunks of 128
RROWS = S // 2  # 1024 rows finalized per core


def _ln_stats(nc, pool, x_ap, eps_sb):
    """bn_stats/bn_aggr mean+rstd for a [128, D] fp32 tile. Returns mv tile;
    mv[:,0:1]=mean, mv[:,1:2]=rstd (after rsqrt)."""
    stats = pool.tile([NP, 2, 6], F32, tag="lnstats")
    nc.vector.bn_stats(out=stats[:, 0, :], in_=x_ap[:, 0:512])
    nc.vector.bn_stats(out=stats[:, 1, :], in_=x_ap[:, 512:1024])
    mv = pool.tile([NP, 2], F32, tag="lnmv")
    nc.vector.bn_aggr(out=mv[:], in_=stats[:])
    nc.scalar.activation(out=mv[:, 1:2], in_=mv[:, 1:2], func=AF.Sqrt,
                         bias=eps_sb[:], scale=1.0)
    nc.vector.reciprocal(out=mv[:, 1:2], in_=mv[:, 1:2])
    return mv


def build(apply_ln1_affine=False, apply_ln2_affine=False):
    nc = bacc.Bacc("TRN2", num_devices=8)

    # ---- DRAM parameters (per-core shards, laid out host-side) ----
    x_p = nc.declare_dram_parameter("x", [S, D], F32, isOutput=False)
    xres_p = nc.declare_dram_parameter("x_res", [RROWS, D], F32, isOutput=False)
    wq_p = nc.declare_dram_parameter("wq", [NP, DC, 512], BF16, isOutput=False)
    wk_p = nc.declare_dram_parameter("wk", [NP, DC, 512], BF16, isOutput=False)
    wv_p = nc.declare_dram_parameter("wv", [NP, DC, 512], BF16, isOutput=False)
    wproj_p = nc.declare_dram_parameter("wproj", [NP, 4, D], BF16, isOutput=False)
    wff1_p = nc.declare_dram_parameter("wff1", [FC, NP, DC * NP], BF16, isOutput=False)
    wff2_p = nc.declare_dram_parameter("wff2", [FC, NP, D], BF16, isOutput=False)
    bproj_p = nc.declare_dram_parameter("bproj", [1, D], F32, isOutput=False)
    bff1_p = nc.declare_dram_parameter("bff1", [NP, FC], F32, isOutput=False)
    bff2_p = nc.declare_dram_parameter("bff2", [1, D], F32, isOutput=False)
    ident_p = nc.declare_dram_parameter("ident", [NP, NP], BF16, isOutput=False)
    mask_p = nc.declare_dram_parameter("mask", [NP, NP], BF16, isOutput=False)
    if apply_ln1_affine:
        ln1w_p = nc.declare_dram_parameter("ln1w", [1, D], F32, isOutput=False)
        ln1b_p = nc.declare_dram_parameter("ln1b", [1, D], F32, isOutput=False)
    if apply_ln2_affine:
        ln2w_p = nc.declare_dram_parameter("ln2w", [1, D], F32, isOutput=False)
        ln2b_p = nc.declare_dram_parameter("ln2b", [1, D], F32, isOutput=False)
    out_p = nc.declare_dram_parameter("out", [RROWS, D], F32, isOutput=True)

    with tile.TileContext(nc) as tc:
        with contextlib.ExitStack() as stack:
            # ---------------- persistent pools ----------------
            const = stack.enter_context(tc.tile_pool(name="const", bufs=1))
            tp_ps = stack.enter_context(tc.tile_pool(name="tp_ps", bufs=2, space="PSUM"))
            dram = stack.enter_context(tc.tile_pool(name="dram", bufs=1, space="DRAM"))
            ctxT_pool = stack.enter_context(tc.tile_pool(name="ctxT", bufs=1))
            wp_pool = stack.enter_context(tc.tile_pool(name="wp", bufs=1))
            small = stack.enter_context(tc.tile_pool(name="small", bufs=2))

            eps_sb = const.tile([NP, 1], F32)
            nc.vector.memset(eps_sb[:], EPS)
            ones_f32 = const.tile([NP, 64], F32)
            nc.vector.memset(ones_f32[:], 1.0)
            ones_sb = const.tile([NP, 64], F32R)
            nc.vector.tensor_copy(out=ones_sb[:], in_=ones_f32[:])
            ident_sb = const.tile([NP, NP], BF16)
            nc.sync.dma_start(out=ident_sb[:], in_=ident_p[:, :])
            mask_sb = const.tile([NP, NP], BF16)
            nc.sync.dma_start(out=mask_sb[:], in_=mask_p[:, :])
            bproj_bc = const.tile([NP, D], F32)
            nc.gpsimd.dma_start(out=bproj_bc[:], in_=bproj_p[:, :].to_broadcast((NP, D)))
            bff2_bc = const.tile([NP, D], F32)
            nc.gpsimd.dma_start(out=bff2_bc[:], in_=bff2_p[:, :].to_broadcast((NP, D)))
            bff1_sb = const.tile([NP, FC], F32)
            nc.sync.dma_start(out=bff1_sb[:], in_=bff1_p[:, :])
            if apply_ln1_affine:
                ln1w_bc = const.tile([NP, D], F32)
                nc.gpsimd.dma_start(out=ln1w_bc[:], in_=ln1w_p[:, :].to_broadcast((NP, D)))
                ln1b_bc = const.tile([NP, D], F32)
                nc.gpsimd.dma_start(out=ln1b_bc[:], in_=ln1b_p[:, :].to_broadcast((NP, D)))
            if apply_ln2_affine:
                ln2w_bc = const.tile([NP, D], F32)
                nc.gpsimd.dma_start(out=ln2w_bc[:], in_=ln2w_p[:, :].to_broadcast((NP, D)))
                ln2b_bc = const.tile([NP, D], F32)
                nc.gpsimd.dma_start(out=ln2b_bc[:], in_=ln2b_p[:, :].to_broadcast((NP, D)))

            # bf16 collective buffers, 2 chunks of 1024 rows each
            cc_in0 = dram.tile([RROWS, D], BF16)
            cc_in1 = dram.tile([RROWS, D], BF16)
            cc_out0 = dram.tile([RROWS // 2, D], BF16)
            cc_out1 = dram.tile([RROWS // 2, D], BF16)

            ctxT = [ctxT_pool.tile([NP, S], BF16, tag=f"ctxT{p}", name=f"ctxT{p}") for p in range(4)]
            wp_sb = wp_pool.tile([NP, 4, D], BF16)

            # ---------------- attention-lifetime pools ----------------
            with contextlib.ExitStack() as att:
                qkT_pool = att.enter_context(tc.tile_pool(name="qkT", bufs=1))
                vaug_pool = att.enter_context(tc.tile_pool(name="vaug", bufs=1))
                e_pool = att.enter_context(tc.tile_pool(name="e", bufs=6))
                attn_pool = att.enter_context(tc.tile_pool(name="attn", bufs=2))
                sc_ps = att.enter_context(tc.tile_pool(name="sc_ps", bufs=4, space="PSUM"))

                vaug = [vaug_pool.tile([NP, GH, HD + 1], BF16, tag=f"v{sc}", name=f"vaug{sc}") for sc in range(SC)]
                for sc in range(SC):
                    nc.vector.memset(vaug[sc][:, :, HD:HD + 1], 1.0)
                qT = [qkT_pool.tile([NP, S], BF16, tag=f"qT{p}", name=f"qT{p}") for p in range(4)]
                kT = [qkT_pool.tile([NP, S], BF16, tag=f"kT{p}", name=f"kT{p}") for p in range(4)]

                # ---- prep phase: LN1 + hT, v rows, q/k projections ----
                with contextlib.ExitStack() as prep:
                    wqkv = prep.enter_context(tc.tile_pool(name="wqkv", bufs=1))
                    hT_pool = prep.enter_context(tc.tile_pool(name="hT", bufs=1))
                    xln = prep.enter_context(tc.tile_pool(name="xln", bufs=2))
                    hrow = prep.enter_context(tc.tile_pool(name="hrow", bufs=3))
                    lnst = prep.enter_context(tc.tile_pool(name="lnst", bufs=4))
                    qkv_ps = prep.enter_context(tc.tile_pool(name="qkv_ps", bufs=2, space="PSUM"))

                    wq_sb = wqkv.tile([NP, DC, 512], BF16)
                    wk_sb = wqkv.tile([NP, DC, 512], BF16)
                    wv_sb = wqkv.tile([NP, DC, 512], BF16)
                    nc.sync.dma_start(out=wq_sb[:], in_=wq_p[:, :, :])
                    nc.sync.dma_start(out=wk_sb[:], in_=wk_p[:, :, :])
                    nc.sync.dma_start(out=wv_sb[:], in_=wv_p[:, :, :])

                    hT = hT_pool.tile([NP, DC, S], BF16)        # LN1(x)^T

                    # LN1 + transpose h -> hT (copies on the scalar engine;
                    # the vector engine is the prep-phase bottleneck)
                    for sc in range(SC):
                        x_sb = xln.tile([NP, D], F32, tag="x")
                        # two half-row DMAs -> two queues (one 512KB dma_start
                        # is a single ~22GB/s queue = 23us on the critical path)
                        for st in range(2):
                            nc.sync.dma_start(
                                out=x_sb[:, st * 512:(st + 1) * 512],
                                in_=x_p[sc * NP:(sc + 1) * NP, st * 512:(st + 1) * 512])
                        mv = _ln_stats(nc, lnst, x_sb[:], eps_sb)
                        h_sb = hrow.tile([NP, D], BF16, tag="h")
                        nc.vector.tensor_scalar(
                            out=h_sb[:], in0=x_sb[:],
                            scalar1=mv[:, 0:1], scalar2=mv[:, 1:2],
                            op0=OP.subtract, op1=OP.mult)
                        if apply_ln1_affine:
                            nc.vector.tensor_mul(out=h_sb[:], in0=h_sb[:], in1=ln1w_bc[:])
                            nc.vector.tensor_add(out=h_sb[:], in0=h_sb[:], in1=ln1b_bc[:])
                        for dc in range(DC):
                            t_ps = tp_ps.tile([NP, NP], BF16, tag="tp")
                            nc.tensor.transpose(t_ps[:], h_sb[:, dc * NP:(dc + 1) * NP], ident_sb[:])
                            dst = hT[:, dc, sc * NP:(sc + 1) * NP]
                            if dc % 2 == 0:
                                nc.scalar.copy(out=dst, in_=t_ps[:])
                            else:
                                nc.vector.tensor_copy(out=dst, in_=t_ps[:])

                    # v rows: per seq chunk, out [128 seq, 512 all heads]
                    for sc in range(SC):
                        ps = qkv_ps.tile([NP, 512], F32, tag="qkvps")
                        for dc in range(DC):
                            nc.tensor.matmul(
                                ps[:],
                                hT[:, dc, sc * NP:(sc + 1) * NP],
                                wv_sb[:, dc, :],
                                start=(dc == 0), stop=(dc == DC - 1))
                        nc.vector.tensor_copy(
                            out=vaug[sc][:, :, 0:HD],
                            in_=ps[:].rearrange("p (h d) -> p h d", h=GH))

                    # q/k projections for ALL pairs
                    for hp in range(4):
                        for s4 in range(QC):
                            cols = slice(s4 * 512, (s4 + 1) * 512)
                            for w_sb, dst in ((wq_sb, qT[hp]), (wk_sb, kT[hp])):
                                ps = qkv_ps.tile([NP, 512], F32, tag="qkvps")
                                for dc in range(DC):
                                    nc.tensor.matmul(
                                        ps[:],
                                        w_sb[:, dc, hp * NP:(hp + 1) * NP],
                                        hT[:, dc, cols],
                                        start=(dc == 0), stop=(dc == DC - 1))
                                if s4 % 2 == 0:
                                    nc.vector.tensor_copy(out=dst[:, cols], in_=ps[:])
                                else:
                                    nc.scalar.copy(out=dst[:, cols], in_=ps[:])

                # wproj load deferred out of the startup DMA window (first
                # needed by emit_proj at qc==1, ~100us later)
                nc.sync.dma_start(out=wp_sb[:], in_=wproj_p[:, :, :])

                # ---- attention: qc-major, both heads of a pair in flight ----
                with contextlib.ExitStack() as att2:
                    ctx_ps_pool = att2.enter_context(tc.tile_pool(name="ctx_ps", bufs=1, space="PSUM"))

                    def emit_one_proj(qs, cc_dst, r):
                        attn_sb = attn_pool.tile([NP, D], BF16, tag="attnsb", name="attn_sb")
                        for nch in range(2):
                            a_ps = sc_ps.tile([NP, 512], F32, tag="sc", name="a_ps")
                            for pair in range(4):
                                nc.tensor.matmul(
                                    a_ps[:],
                                    ctxT[pair][:, qs * NP:(qs + 1) * NP],
                                    wp_sb[:, pair, nch * 512:(nch + 1) * 512],
                                    start=(pair == 0), stop=(pair == 3))
                            nc.vector.tensor_copy(out=attn_sb[:, nch * 512:(nch + 1) * 512], in_=a_ps[:])
                        nc.sync.dma_start(out=cc_dst[r * NP:(r + 1) * NP, :], in_=attn_sb[:])

                    def emit_proj(qs_lo, qs_hi, cc_dst):
                        for qs in range(qs_lo, qs_hi):
                            emit_one_proj(qs, cc_dst, qs - qs_lo)

                    # proj chains for rows 0-1023, interleaved one-per-step
                    # into qc==2's attention so the scalar exp stream is
                    # never starved by a block of PE-only proj work
                    pending_proj = []

                    pending_norm = []

                    def flush_norm():
                        while pending_norm:
                            pending_norm.pop(0)()

                    def make_norm(ctx_ps, hp, po, qbase):
                        def emit():
                            den = small.tile([NP, 512], F32R, tag="den", name="den")
                            nc.vector.tensor_copy(out=den[64:65, :], in_=ctx_ps[64:65, :])
                            b_ps = sc_ps.tile([64, 512], F32, tag="sc", name="b_ps")
                            nc.tensor.matmul(b_ps[:], ones_sb[64:65, :], den[64:65, :],
                                             start=True, stop=True)
                            b_sb = small.tile([64, 512], F32, tag="bsb", name="b_sb")
                            nc.vector.reciprocal_approx_fast(out=b_sb[:], in_=b_ps[:])
                            nc.vector.tensor_mul(
                                out=ctxT[hp][po:po + 64, qbase:qbase + 512],
                                in0=ctx_ps[0:64, :], in1=b_sb[:])
                        return emit

                    for qc in range(QC):
                        qbase = qc * 512
                        kcs = [4 * qc] + list(range(0, 4 * qc)) + [4 * qc + 1, 4 * qc + 2, 4 * qc + 3]
                        for hp in range(4):
                            ctxA = ctx_ps_pool.tile([HD + 1, 512], F32, tag="ctxA", name="ctxA")
                            ctxB = ctx_ps_pool.tile([HD + 1, 512], F32, tag="ctxB", name="ctxB")
                            pend = None
                            for i, kc in enumerate(kcs):
                                off = max(0, 128 * kc - qbase)
                                sA = sc_ps.tile([NP, 512], F32, tag="sc", name="sA")
                                sB = sc_ps.tile([NP, 512], F32, tag="sc", name="sB")
                                # concurrent in the PE array: row groups 0-63 / 64-127
                                nc.tensor.matmul(
                                    sA[:, off:512],
                                    kT[hp][0:64, kc * NP:(kc + 1) * NP],
                                    qT[hp][0:64, qbase + off:qbase + 512],
                                    start=True, stop=True)
                                nc.tensor.matmul(
                                    sB[:, off:512],
                                    kT[hp][64:128, kc * NP:(kc + 1) * NP],
                                    qT[hp][64:128, qbase + off:qbase + 512],
                                    start=True, stop=True)
                                eA = e_pool.tile([NP, 512], BF16, tag="esb", name="eA")
                                eB = e_pool.tile([NP, 512], BF16, tag="esb", name="eB")
                                nc.scalar.activation(out=eA[:, off:512], in_=sA[:, off:512], func=AF.Exp)
                                nc.scalar.activation(out=eB[:, off:512], in_=sB[:, off:512], func=AF.Exp)
                                if 4 * qc <= kc:
                                    nc.vector.tensor_mul(
                                        out=eA[:, off:off + 128], in0=eA[:, off:off + 128], in1=mask_sb[:])
                                    nc.vector.tensor_mul(
                                        out=eB[:, off:off + 128], in0=eB[:, off:off + 128], in1=mask_sb[:])
                                if i == 0:
                                    # previous pair's softmax normalization, deferred
                                    # past this pair's first scores/exps so the
                                    # scalar-engine exp stream never stalls at the
                                    # pair boundary (must drain before this pair's
                                    # first ctx matmul reuses the ctx banks)
                                    flush_norm()
                                elif pending_proj:
                                    pending_proj.pop(0)()
                                if pend is not None:
                                    off_, eA_, eB_, kc_, first_ = pend
                                    nc.tensor.matmul(
                                        ctxA[:, off_:512], vaug[kc_][:, 2 * hp, :],
                                        eA_[:, off_:512], start=first_, stop=False)
                                    nc.tensor.matmul(
                                        ctxB[:, off_:512], vaug[kc_][:, 2 * hp + 1, :],
                                        eB_[:, off_:512], start=first_, stop=False)
                                pend = (off, eA, eB, kc, i == 0)
                            off_, eA_, eB_, kc_, first_ = pend
                            nc.tensor.matmul(
                                ctxA[:, off_:512], vaug[kc_][:, 2 * hp, :],
                                eA_[:, off_:512], start=first_, stop=True)
                            nc.tensor.matmul(
                                ctxB[:, off_:512], vaug[kc_][:, 2 * hp + 1, :],
                                eB_[:, off_:512], start=first_, stop=True)
                            pending_norm.append(make_norm(ctxA, hp, 0, qbase))
                            pending_norm.append(make_norm(ctxB, hp, 64, qbase))

                        if qc == 1:
                            pending_proj.extend(
                                (lambda qs=qs: emit_one_proj(qs, cc_in0, qs))
                                for qs in range(8))
                        if qc == 2:
                            while pending_proj:
                                pending_proj.pop(0)()
                            nc.gpsimd.collective_compute(
                                "ReduceScatter", OP.add,
                                replica_groups=[[0, 1], [2, 3], [4, 5], [6, 7]],
                                ins=[cc_in0[:].opt()], outs=[cc_out0[:].opt()])
                            # qs 8-11 (rows 1024-1535) only need qc<=2 context;
                            # interleave them into qc==3's attention steps
                            pending_proj.extend(
                                (lambda qs=qs: emit_one_proj(qs, cc_in1, qs - 8))
                                for qs in range(8, 12))
                        if qc == 3:
                            while pending_proj:
                                pending_proj.pop(0)()
                            flush_norm()
                            for qs in range(12, 16):
                                emit_one_proj(qs, cc_in1, qs - 8)

            # RS#2 issued OUTSIDE the attention pool scopes: the pool-stack
            # close drains all engines, so a collective issued inside would
            # serialize the whole FFN behind its completion.
            nc.gpsimd.collective_compute(
                "ReduceScatter", OP.add,
                replica_groups=[[0, 1], [2, 3], [4, 5], [6, 7]],
                ins=[cc_in1[:].opt()], outs=[cc_out1[:].opt()])

            # ---------------- FFN phase (1024 rows per core) ----------------
            with contextlib.ExitStack() as ffn:
                y_pool = ffn.enter_context(tc.tile_pool(name="y", bufs=1))
                y2T_pool = ffn.enter_context(tc.tile_pool(name="y2T", bufs=1))
                g_pool = ffn.enter_context(tc.tile_pool(name="g", bufs=64))
                yw = ffn.enter_context(tc.tile_pool(name="yw", bufs=3))
                lnst2 = ffn.enter_context(tc.tile_pool(name="lnst2", bufs=4))
                w1_pool = ffn.enter_context(tc.tile_pool(name="w1", bufs=5))
                w2_pool = ffn.enter_context(tc.tile_pool(name="w2", bufs=5))
                out_pool = ffn.enter_context(tc.tile_pool(name="outp", bufs=3))
                ff_ps_pool = ffn.enter_context(tc.tile_pool(name="ff_ps", bufs=2, space="PSUM"))
                z_ps_pool = ffn.enter_context(tc.tile_pool(name="z_ps", bufs=1, space="PSUM"))

                y = [y_pool.tile([NP, D], F32, tag=f"y{rc}", name=f"y{rc}") for rc in range(8)]
                y2T = y2T_pool.tile([NP, DC, RROWS], BF16)

                cc_outs = (cc_out0, cc_out1)
                for half in range(2):
                    # residual + LN2 for this half only, so half 0's FFN can
                    # run while the second ReduceScatter is still in flight
                    for r4 in range(4):
                        rc = half * 4 + r4
                        rs_sb = yw.tile([NP, D], BF16, tag="rs")
                        for st in range(2):
                            nc.sync.dma_start(
                                out=rs_sb[:, st * 512:(st + 1) * 512],
                                in_=cc_outs[half][r4 * NP:(r4 + 1) * NP, st * 512:(st + 1) * 512])
                        xr_sb = yw.tile([NP, D], F32, tag="xr")
                        nc.sync.dma_start(out=xr_sb[:], in_=xres_p[rc * NP:(rc + 1) * NP, :])
                        nc.vector.tensor_copy(out=y[rc][:], in_=rs_sb[:])
                        nc.vector.tensor_add(out=y[rc][:], in0=y[rc][:], in1=xr_sb[:])
                        nc.vector.tensor_add(out=y[rc][:], in0=y[rc][:], in1=bproj_bc[:])
                        mv = _ln_stats(nc, lnst2, y[rc][:], eps_sb)
                        y2_sb = yw.tile([NP, D], BF16, tag="y2")
                        nc.vector.tensor_scalar(
                            out=y2_sb[:], in0=y[rc][:],
                            scalar1=mv[:, 0:1], scalar2=mv[:, 1:2],
                            op0=OP.subtract, op1=OP.mult)
                        if apply_ln2_affine:
                            nc.vector.tensor_mul(out=y2_sb[:], in0=y2_sb[:], in1=ln2w_bc[:])
                            nc.vector.tensor_add(out=y2_sb[:], in0=y2_sb[:], in1=ln2b_bc[:])
                        for dc in range(DC):
                            t_ps = tp_ps.tile([NP, NP], BF16, tag="tp")
                            nc.tensor.transpose(t_ps[:], y2_sb[:, dc * NP:(dc + 1) * NP], ident_sb[:])
                            nc.scalar.copy(out=y2T[:, dc, rc * NP:(rc + 1) * NP], in_=t_ps[:])

                    hcols = slice(half * 512, (half + 1) * 512)
                    gts = []
                    for ffc in range(FC):
                        w1_sb = w1_pool.tile([NP, DC * NP], BF16, tag="w1")
                        nc.sync.dma_start(out=w1_sb[:], in_=wff1_p[ffc, :, :])
                        ff_ps = ff_ps_pool.tile([NP, 512], F32, tag="ffps")
                        for dc in range(DC):
                            nc.tensor.matmul(
                                ff_ps[:],
                                w1_sb[:, dc * NP:(dc + 1) * NP],
                                y2T[:, dc, hcols],
                                start=(dc == 0), stop=(dc == DC - 1))
                        g_sb = g_pool.tile([NP, 512], BF16, tag="g")
                        nc.scalar.activation(out=g_sb[:], in_=ff_ps[:], func=AF.Gelu,
                                             bias=bff1_sb[:, ffc:ffc + 1], scale=1.0)
                        gts.append(g_sb)
                    for nch in range(2):
                        ncols = slice(nch * 512, (nch + 1) * 512)
                        z_pss = [z_ps_pool.tile([NP, 512], F32, tag=f"zps{r}", name=f"zps{r}") for r in range(4)]
                        for ffc in range(FC):
                            w2_sb = w2_pool.tile([NP, 512], BF16, tag="w2")
                            nc.sync.dma_start(out=w2_sb[:], in_=wff2_p[ffc, :, ncols])
                            for r4 in range(4):
                                nc.tensor.matmul(
                                    z_pss[r4][:],
                                    gts[ffc][:, r4 * NP:(r4 + 1) * NP],
                                    w2_sb[:],
                                    start=(ffc == 0), stop=(ffc == FC - 1))
                        for r4 in range(4):
                            rc = half * 4 + r4
                            o_sb = out_pool.tile([NP, 512], F32, tag="osb")
                            nc.vector.tensor_add(out=o_sb[:], in0=z_pss[r4][:], in1=y[rc][:, ncols])
                            nc.vector.tensor_add(out=o_sb[:], in0=o_sb[:], in1=bff2_bc[:, ncols])
                            nc.sync.dma_start(out=out_p[rc * NP:(rc + 1) * NP, ncols], in_=o_sb[:])

    nc.compile()
    return nc


# ------------------------- host-side driver -------------------------

_BF = ml_dtypes.bfloat16


def _core_rows(g):
    return np.r_[512 * g:512 * g + 512, 1024 + 512 * g:1536 + 512 * g]


def _prep_core_inputs(inputs, b, g):
    x = np.asarray(inputs["x"], np.float32)
    w_qkv = np.asarray(inputs["w_qkv"], np.float32).reshape(D, H, HD, 3)
    hs = slice(g * GH, (g + 1) * GH)
    w_k = w_qkv[:, hs, :, 0].reshape(D, GH * HD)
    w_q = (w_qkv[:, hs, :, 1] * (HD ** -0.5)).reshape(D, GH * HD)
    w_v = w_qkv[:, hs, :, 2].reshape(D, GH * HD)

    def tile_kxm(w):  # [D, 512] -> [128, DC, 512]
        return np.ascontiguousarray(w.reshape(DC, NP, GH * HD).transpose(1, 0, 2))

    w_proj = np.asarray(inputs["w_proj"], np.float32)
    wp = np.ascontiguousarray(
        w_proj[g * 512:(g + 1) * 512, :].reshape(4, NP, D).transpose(1, 0, 2))

    w_ff1 = np.asarray(inputs["w_ff1"], np.float32)
    w1t = np.ascontiguousarray(
        w_ff1.reshape(DC, NP, FC, NP).transpose(2, 1, 0, 3).reshape(FC, NP, DC * NP))
    w_ff2 = np.asarray(inputs["w_ff2"], np.float32)
    w2t = np.ascontiguousarray(w_ff2.reshape(FC, NP, D))

    j = np.arange(NP)[:, None]
    i = np.arange(NP)[None, :]
    mask = (j <= i).astype(np.float32)

    return {
        "x": np.ascontiguousarray(x[b]),
        "x_res": np.ascontiguousarray(x[b][_core_rows(g)]),
        "wq": tile_kxm(w_q).astype(_BF),
        "wk": tile_kxm(w_k).astype(_BF),
        "wv": tile_kxm(w_v).astype(_BF),
        "wproj": wp.astype(_BF),
        "wff1": w1t.astype(_BF),
        "wff2": w2t.astype(_BF),
        "bproj": np.asarray(inputs["b_proj"], np.float32).reshape(1, D).copy(),
        "bff1": np.ascontiguousarray(np.asarray(inputs["b_ff1"], np.float32).reshape(FC, NP).T),
        "bff2": np.asarray(inputs["b_ff2"], np.float32).reshape(1, D).copy(),
        "ident": np.eye(NP, dtype=_BF),
        "mask": mask.astype(_BF),
    }


_NC_CACHE = {}


def kernel(**inputs):
    ln1w = np.asarray(inputs["ln1_w"], np.float32)
    ln1b = np.asarray(inputs["ln1_b"], np.float32)
    ln2w = np.asarray(inputs["ln2_w"], np.float32)
    ln2b = np.asarray(inputs["ln2_b"], np.float32)
    a1 = not (np.allclose(ln1w, 1.0) and np.allclose(ln1b, 0.0))
    a2 = not (np.allclose(ln2w, 1.0) and np.allclose(ln2b, 0.0))

    key = (a1, a2)
    if key not in _NC_CACHE:
        _NC_CACHE[key] = build(apply_ln1_affine=a1, apply_ln2_affine=a2)
    nc = _NC_CACHE[key]

    in_maps = []
    for core in range(8):
        b, g = core // 2, core % 2
        m = _prep_core_inputs(inputs, b, g)
        if a1:
            m["ln1w"] = ln1w.reshape(1, D).copy()
            m["ln1b"] = ln1b.reshape(1, D).copy()
        if a2:
            m["ln2w"] = ln2w.reshape(1, D).copy()
            m["ln2b"] = ln2b.reshape(1, D).copy()
        in_maps.append(m)

    res = run_bass_kernel_spmd(nc, in_maps, core_ids=list(range(8)))

    out = np.empty((B, S, D), np.float32)
    for core in range(8):
        b, g = core // 2, core % 2
        out[b][_core_rows(g)] = res.results[core]["out"]
    return out


# revision 29
# speedup vs baseline: 1.0727x; 1.0221x over previous
"""Trainium2 Bass kernel for a pre-LN transformer block (attention + FFN).

Sharding over 8 NeuronCores: core (b, g) = batch b (0..3) x head-group g (0..1).
Each core runs LN1 + QKV (its 8 heads) + causal attention + its slice of the
output projection for its batch; a pairwise bf16 ReduceScatter (2 chunks,
first overlapped with attention of the second query half) sums the two
head-groups' partial attn_out; each core then finishes 1024 rows
(residual + LN2 + full FFN).

Attention inner loop: the two heads of a q/k pair live on partitions 0:64 and
64:128, so their K=64 score matmuls run CONCURRENTLY in the PE array via
row-group tiling (tile_position (0,0)/(64,0)); a 1-step software pipeline
issues scores(i)/exp(i) before ctx(i-1) so the PE never stalls on the scalar
engine and the HAM clock stays warm. Softmax denominators via a ones-column
on V and reciprocal_approx_fast.
"""
import sys

if "/opt/trn_rl_repo" not in sys.path:
    sys.path.insert(0, "/opt/trn_rl_repo")

import contextlib

import numpy as np
import ml_dtypes

import concourse.bass as bass
import concourse.bacc as bacc
import concourse.tile as tile
from concourse import mybir
from concourse.bass_utils import run_bass_kernel_spmd

F32 = mybir.dt.float32
F32R = mybir.dt.float32r
BF16 = mybir.dt.bfloat16
AF = mybir.ActivationFunctionType
OP = mybir.AluOpType

B, S, D, H = 4, 2048, 1024, 16
HD = D // H
FF = 4 * D
EPS = 1e-5
GH = 8          # heads per core
NP = 128        # partitions
SC = S // NP    # 16 seq chunks of 128
DC = D // NP    # 8 d-chunks
QC = S // 512   # 4 q-chunks of 512
FC = FF // NP   # 32 ff chunks of 128
RROWS = S // 2  # 1024 rows finalized per core


def _ln_stats(nc, pool, x_ap, eps_sb):
    """bn_stats/bn_aggr mean+rstd for a [128, D] fp32 tile. Returns mv tile;
    mv[:,0:1]=mean, mv[:,1:2]=rstd (after rsqrt)."""
    stats = pool.tile([NP, 2, 6], F32, tag="lnstats")
    nc.vector.bn_stats(out=stats[:, 0, :], in_=x_ap[:, 0:512])
    nc.vector.bn_stats(out=stats[:, 1, :], in_=x_ap[:, 512:1024])
    mv = pool.tile([NP, 2], F32, tag="lnmv")
    nc.vector.bn_aggr(out=mv[:], in_=stats[:])
    nc.scalar.activation(out=mv[:, 1:2], in_=mv[:, 1:2], func=AF.Sqrt,
                         bias=eps_sb[:], scale=1.0)
    nc.vector.reciprocal(out=mv[:, 1:2], in_=mv[:, 1:2])
    return mv


def build(apply_ln1_affine=False, apply_ln2_affine=False):
    nc = bacc.Bacc("TRN2", num_devices=8)

    # ---- DRAM parameters (per-core shards, laid out host-side) ----
    x_p = nc.declare_dram_parameter("x", [S, D], F32, isOutput=False)
    xres_p = nc.declare_dram_parameter("x_res", [RROWS, D], F32, isOutput=False)
    wq_p = nc.declare_dram_parameter("wq", [NP, DC, 512], BF16, isOutput=False)
    wk_p = nc.declare_dram_parameter("wk", [NP, DC, 512], BF16, isOutput=False)
    wv_p = nc.declare_dram_parameter("wv", [NP, DC, 512], BF16, isOutput=False)
    wproj_p = nc.declare_dram_parameter("wproj", [NP, 4, D], BF16, isOutput=False)
    wff1_p = nc.declare_dram_parameter("wff1", [FC, NP, DC * NP], BF16, isOutput=False)
    wff2_p = nc.declare_dram_parameter("wff2", [FC, NP, D], BF16, isOutput=False)
    bproj_p = nc.declare_dram_parameter("bproj", [1, D], F32, isOutput=False)
    bff1_p = nc.declare_dram_parameter("bff1", [NP, FC], F32, isOutput=False)
    bff2_p = nc.declare_dram_parameter("bff2", [1, D], F32, isOutput=False)
    ident_p = nc.declare_dram_parameter("ident", [NP, NP], BF16, isOutput=False)
    mask_p = nc.declare_dram_parameter("mask", [NP, NP], BF16, isOutput=False)
    if apply_ln1_affine:
        ln1w_p = nc.declare_dram_parameter("ln1w", [1, D], F32, isOutput=False)
        ln1b_p = nc.declare_dram_parameter("ln1b", [1, D], F32, isOutput=False)
    if apply_ln2_affine:
        ln2w_p = nc.declare_dram_parameter("ln2w", [1, D], F32, isOutput=False)
        ln2b_p = nc.declare_dram_parameter("ln2b", [1, D], F32, isOutput=False)
    out_p = nc.declare_dram_parameter("out", [RROWS, D], F32, isOutput=True)

    with tile.TileContext(nc) as tc:
        with contextlib.ExitStack() as stack:
            # ---------------- persistent pools ----------------
            const = stack.enter_context(tc.tile_pool(name="const", bufs=1))
            tp_ps = stack.enter_context(tc.tile_pool(name="tp_ps", bufs=2, space="PSUM"))
            dram = stack.enter_context(tc.tile_pool(name="dram", bufs=1, space="DRAM"))
            ctxT_pool = stack.enter_context(tc.tile_pool(name="ctxT", bufs=1))
            wp_pool = stack.enter_context(tc.tile_pool(name="wp", bufs=1))
            small = stack.enter_context(tc.tile_pool(name="small", bufs=2))

            eps_sb = const.tile([NP, 1], F32)
            nc.vector.memset(eps_sb[:], EPS)
            ones_f32 = const.tile([NP, 64], F32)
            nc.vector.memset(ones_f32[:], 1.0)
            ones_sb = const.tile([NP, 64], F32R)
            nc.vector.tensor_copy(out=ones_sb[:], in_=ones_f32[:])
            ident_sb = const.tile([NP, NP], BF16)
            nc.sync.dma_start(out=ident_sb[:], in_=ident_p[:, :])
            mask_sb = const.tile([NP, NP], BF16)
            nc.sync.dma_start(out=mask_sb[:], in_=mask_p[:, :])
            bproj_bc = const.tile([NP, D], F32)
            nc.gpsimd.dma_start(out=bproj_bc[:], in_=bproj_p[:, :].to_broadcast((NP, D)))
            bff2_bc = const.tile([NP, D], F32)
            nc.gpsimd.dma_start(out=bff2_bc[:], in_=bff2_p[:, :].to_broadcast((NP, D)))
            bff1_sb = const.tile([NP, FC], F32)
            nc.sync.dma_start(out=bff1_sb[:], in_=bff1_p[:, :])
            if apply_ln1_affine:
                ln1w_bc = const.tile([NP, D], F32)
                nc.gpsimd.dma_start(out=ln1w_bc[:], in_=ln1w_p[:, :].to_broadcast((NP, D)))
                ln1b_bc = const.tile([NP, D], F32)
                nc.gpsimd.dma_start(out=ln1b_bc[:], in_=ln1b_p[:, :].to_broadcast((NP, D)))
            if apply_ln2_affine:
                ln2w_bc = const.tile([NP, D], F32)
                nc.gpsimd.dma_start(out=ln2w_bc[:], in_=ln2w_p[:, :].to_broadcast((NP, D)))
                ln2b_bc = const.tile([NP, D], F32)
                nc.gpsimd.dma_start(out=ln2b_bc[:], in_=ln2b_p[:, :].to_broadcast((NP, D)))

            # bf16 collective buffers, 2 chunks of 1024 rows each
            cc_in0 = dram.tile([RROWS, D], BF16)
            cc_in1 = dram.tile([RROWS, D], BF16)
            cc_out0 = dram.tile([RROWS // 2, D], BF16)
            cc_out1 = dram.tile([RROWS // 2, D], BF16)

            ctxT = [ctxT_pool.tile([NP, S], BF16, tag=f"ctxT{p}", name=f"ctxT{p}") for p in range(4)]
            wp_sb = wp_pool.tile([NP, 4, D], BF16)

            # ---------------- attention-lifetime pools ----------------
            with contextlib.ExitStack() as att:
                qkT_pool = att.enter_context(tc.tile_pool(name="qkT", bufs=1))
                vaug_pool = att.enter_context(tc.tile_pool(name="vaug", bufs=1))
                e_pool = att.enter_context(tc.tile_pool(name="e", bufs=6))
                attn_pool = att.enter_context(tc.tile_pool(name="attn", bufs=2))
                sc_ps = att.enter_context(tc.tile_pool(name="sc_ps", bufs=4, space="PSUM"))

                vaug = [vaug_pool.tile([NP, GH, HD + 1], BF16, tag=f"v{sc}", name=f"vaug{sc}") for sc in range(SC)]
                for sc in range(SC):
                    nc.vector.memset(vaug[sc][:, :, HD:HD + 1], 1.0)
                qT = [qkT_pool.tile([NP, S], BF16, tag=f"qT{p}", name=f"qT{p}") for p in range(4)]
                kT = [qkT_pool.tile([NP, S], BF16, tag=f"kT{p}", name=f"kT{p}") for p in range(4)]

                # ---- prep phase: LN1 + hT, v rows, q/k projections ----
                with contextlib.ExitStack() as prep:
                    wqkv = prep.enter_context(tc.tile_pool(name="wqkv", bufs=1))
                    hT_pool = prep.enter_context(tc.tile_pool(name="hT", bufs=1))
                    xln = prep.enter_context(tc.tile_pool(name="xln", bufs=2))
                    hrow = prep.enter_context(tc.tile_pool(name="hrow", bufs=3))
                    lnst = prep.enter_context(tc.tile_pool(name="lnst", bufs=4))
                    qkv_ps = prep.enter_context(tc.tile_pool(name="qkv_ps", bufs=2, space="PSUM"))

                    wq_sb = wqkv.tile([NP, DC, 512], BF16)
                    wk_sb = wqkv.tile([NP, DC, 512], BF16)
                    wv_sb = wqkv.tile([NP, DC, 512], BF16)
                    nc.sync.dma_start(out=wq_sb[:], in_=wq_p[:, :, :])
                    nc.sync.dma_start(out=wk_sb[:], in_=wk_p[:, :, :])
                    nc.sync.dma_start(out=wv_sb[:], in_=wv_p[:, :, :])

                    hT = hT_pool.tile([NP, DC, S], BF16)        # LN1(x)^T

                    # LN1 + transpose h -> hT (copies on the scalar engine;
                    # the vector engine is the prep-phase bottleneck)
                    for sc in range(SC):
                        x_sb = xln.tile([NP, D], F32, tag="x")
                        # two half-row DMAs -> two queues (one 512KB dma_start
                        # is a single ~22GB/s queue = 23us on the critical path)
                        for st in range(2):
                            nc.sync.dma_start(
                                out=x_sb[:, st * 512:(st + 1) * 512],
                                in_=x_p[sc * NP:(sc + 1) * NP, st * 512:(st + 1) * 512])
                        mv = _ln_stats(nc, lnst, x_sb[:], eps_sb)
                        h_sb = hrow.tile([NP, D], BF16, tag="h")
                        nc.vector.tensor_scalar(
                            out=h_sb[:], in0=x_sb[:],
                            scalar1=mv[:, 0:1], scalar2=mv[:, 1:2],
                            op0=OP.subtract, op1=OP.mult)
                        if apply_ln1_affine:
                            nc.vector.tensor_mul(out=h_sb[:], in0=h_sb[:], in1=ln1w_bc[:])
                            nc.vector.tensor_add(out=h_sb[:], in0=h_sb[:], in1=ln1b_bc[:])
                        for dc in range(DC):
                            t_ps = tp_ps.tile([NP, NP], BF16, tag="tp")
                            nc.tensor.transpose(t_ps[:], h_sb[:, dc * NP:(dc + 1) * NP], ident_sb[:])
                            dst = hT[:, dc, sc * NP:(sc + 1) * NP]
                            if dc % 2 == 0:
                                nc.scalar.copy(out=dst, in_=t_ps[:])
                            else:
                                nc.vector.tensor_copy(out=dst, in_=t_ps[:])

                    # v rows: per seq chunk, out [128 seq, 512 all heads]
                    for sc in range(SC):
                        ps = qkv_ps.tile([NP, 512], F32, tag="qkvps")
                        for dc in range(DC):
                            nc.tensor.matmul(
                                ps[:],
                                hT[:, dc, sc * NP:(sc + 1) * NP],
                                wv_sb[:, dc, :],
                                start=(dc == 0), stop=(dc == DC - 1))
                        nc.vector.tensor_copy(
                            out=vaug[sc][:, :, 0:HD],
                            in_=ps[:].rearrange("p (h d) -> p h d", h=GH))

                    # q/k projections for ALL pairs
                    for hp in range(4):
                        for s4 in range(QC):
                            cols = slice(s4 * 512, (s4 + 1) * 512)
                            for w_sb, dst in ((wq_sb, qT[hp]), (wk_sb, kT[hp])):
                                ps = qkv_ps.tile([NP, 512], F32, tag="qkvps")
                                for dc in range(DC):
                                    nc.tensor.matmul(
                                        ps[:],
                                        w_sb[:, dc, hp * NP:(hp + 1) * NP],
                                        hT[:, dc, cols],
                                        start=(dc == 0), stop=(dc == DC - 1))
                                if s4 % 2 == 0:
                                    nc.vector.tensor_copy(out=dst[:, cols], in_=ps[:])
                                else:
                                    nc.scalar.copy(out=dst[:, cols], in_=ps[:])

                # wproj load deferred out of the startup DMA window (first
                # needed by emit_proj at qc==1, ~100us later)
                nc.sync.dma_start(out=wp_sb[:], in_=wproj_p[:, :, :])

                # ---- attention: qc-major, both heads of a pair in flight ----
                with contextlib.ExitStack() as att2:
                    ctx_ps_pool = att2.enter_context(tc.tile_pool(name="ctx_ps", bufs=1, space="PSUM"))

                    def emit_one_proj(qs, cc_dst, r):
                        attn_sb = attn_pool.tile([NP, D], BF16, tag="attnsb", name="attn_sb")
                        for nch in range(2):
                            a_ps = sc_ps.tile([NP, 512], F32, tag="sc", name="a_ps")
                            for pair in range(4):
                                nc.tensor.matmul(
                                    a_ps[:],
                                    ctxT[pair][:, qs * NP:(qs + 1) * NP],
                                    wp_sb[:, pair, nch * 512:(nch + 1) * 512],
                                    start=(pair == 0), stop=(pair == 3))
                            nc.vector.tensor_copy(out=attn_sb[:, nch * 512:(nch + 1) * 512], in_=a_ps[:])
                        nc.sync.dma_start(out=cc_dst[r * NP:(r + 1) * NP, :], in_=attn_sb[:])

                    def emit_proj(qs_lo, qs_hi, cc_dst):
                        for qs in range(qs_lo, qs_hi):
                            emit_one_proj(qs, cc_dst, qs - qs_lo)

                    # proj chains for rows 0-1023, interleaved one-per-step
                    # into qc==2's attention so the scalar exp stream is
                    # never starved by a block of PE-only proj work
                    pending_proj = []

                    pending_norm = []

                    def flush_norm():
                        while pending_norm:
                            pending_norm.pop(0)()

                    def make_norm(ctx_ps, hp, po, qbase):
                        def emit():
                            den = small.tile([NP, 512], F32R, tag="den", name="den")
                            nc.vector.tensor_copy(out=den[64:65, :], in_=ctx_ps[64:65, :])
                            b_ps = sc_ps.tile([64, 512], F32, tag="sc", name="b_ps")
                            nc.tensor.matmul(b_ps[:], ones_sb[64:65, :], den[64:65, :],
                                             start=True, stop=True)
                            b_sb = small.tile([64, 512], F32, tag="bsb", name="b_sb")
                            nc.vector.reciprocal_approx_fast(out=b_sb[:], in_=b_ps[:])
                            nc.vector.tensor_mul(
                                out=ctxT[hp][po:po + 64, qbase:qbase + 512],
                                in0=ctx_ps[0:64, :], in1=b_sb[:])
                        return emit

                    for qc in range(QC):
                        qbase = qc * 512
                        kcs = [4 * qc] + list(range(0, 4 * qc)) + [4 * qc + 1, 4 * qc + 2, 4 * qc + 3]
                        qstep = 0
                        for hp in range(4):
                            ctxA = ctx_ps_pool.tile([HD + 1, 512], F32, tag="ctxA", name="ctxA")
                            ctxB = ctx_ps_pool.tile([HD + 1, 512], F32, tag="ctxB", name="ctxB")
                            pend = None
                            for i, kc in enumerate(kcs):
                                off = max(0, 128 * kc - qbase)
                                sA = sc_ps.tile([NP, 512], F32, tag="sc", name="sA")
                                sB = sc_ps.tile([NP, 512], F32, tag="sc", name="sB")
                                # concurrent in the PE array: row groups 0-63 / 64-127
                                nc.tensor.matmul(
                                    sA[:, off:512],
                                    kT[hp][0:64, kc * NP:(kc + 1) * NP],
                                    qT[hp][0:64, qbase + off:qbase + 512],
                                    start=True, stop=True)
                                nc.tensor.matmul(
                                    sB[:, off:512],
                                    kT[hp][64:128, kc * NP:(kc + 1) * NP],
                                    qT[hp][64:128, qbase + off:qbase + 512],
                                    start=True, stop=True)
                                eA = e_pool.tile([NP, 512], BF16, tag="esb", name="eA")
                                eB = e_pool.tile([NP, 512], BF16, tag="esb", name="eB")
                                nc.scalar.activation(out=eA[:, off:512], in_=sA[:, off:512], func=AF.Exp)
                                nc.scalar.activation(out=eB[:, off:512], in_=sB[:, off:512], func=AF.Exp)
                                if 4 * qc <= kc:
                                    nc.vector.tensor_mul(
                                        out=eA[:, off:off + 128], in0=eA[:, off:off + 128], in1=mask_sb[:])
                                    nc.vector.tensor_mul(
                                        out=eB[:, off:off + 128], in0=eB[:, off:off + 128], in1=mask_sb[:])
                                qstep += 1
                                if i == 0:
                                    # previous pair's softmax normalization, deferred
                                    # past this pair's first scores/exps so the
                                    # scalar-engine exp stream never stalls at the
                                    # pair boundary (must drain before this pair's
                                    # first ctx matmul reuses the ctx banks)
                                    flush_norm()
                                elif pending_proj and qstep % 6 == 0:
                                    # spread deferred proj chains evenly across
                                    # the qc instead of bunching them into the
                                    # first pair's steps
                                    pending_proj.pop(0)()
                                if pend is not None:
                                    off_, eA_, eB_, kc_, first_ = pend
                                    nc.tensor.matmul(
                                        ctxA[:, off_:512], vaug[kc_][:, 2 * hp, :],
                                        eA_[:, off_:512], start=first_, stop=False)
                                    nc.tensor.matmul(
                                        ctxB[:, off_:512], vaug[kc_][:, 2 * hp + 1, :],
                                        eB_[:, off_:512], start=first_, stop=False)
                                pend = (off, eA, eB, kc, i == 0)
                            off_, eA_, eB_, kc_, first_ = pend
                            nc.tensor.matmul(
                                ctxA[:, off_:512], vaug[kc_][:, 2 * hp, :],
                                eA_[:, off_:512], start=first_, stop=True)
                            nc.tensor.matmul(
                                ctxB[:, off_:512], vaug[kc_][:, 2 * hp + 1, :],
                                eB_[:, off_:512], start=first_, stop=True)
                            pending_norm.append(make_norm(ctxA, hp, 0, qbase))
                            pending_norm.append(make_norm(ctxB, hp, 64, qbase))

                        if qc == 1:
                            pending_proj.extend(
                                (lambda qs=qs: emit_one_proj(qs, cc_in0, qs))
                                for qs in range(8))
                        if qc == 2:
                            while pending_proj:
                                pending_proj.pop(0)()
                            nc.gpsimd.collective_compute(
                                "ReduceScatter", OP.add,
                                replica_groups=[[0, 1], [2, 3], [4, 5], [6, 7]],
                                ins=[cc_in0[:].opt()], outs=[cc_out0[:].opt()])
                            # qs 8-11 (rows 1024-1535) only need qc<=2 context;
                            # interleave them into qc==3's attention steps
                            pending_proj.extend(
                                (lambda qs=qs: emit_one_proj(qs, cc_in1, qs - 8))
                                for qs in range(8, 12))
                        if qc == 3:
                            while pending_proj:
                                pending_proj.pop(0)()
                            flush_norm()
                            for qs in range(12, 16):
                                emit_one_proj(qs, cc_in1, qs - 8)

            # RS#2 issued OUTSIDE the attention pool scopes: the pool-stack
            # close drains all engines, so a collective issued inside would
            # serialize the whole FFN behind its completion.
            nc.gpsimd.collective_compute(
                "ReduceScatter", OP.add,
                replica_groups=[[0, 1], [2, 3], [4, 5], [6, 7]],
                ins=[cc_in1[:].opt()], outs=[cc_out1[:].opt()])

            # ---------------- FFN phase (1024 rows per core) ----------------
            with contextlib.ExitStack() as ffn:
                y_pool = ffn.enter_context(tc.tile_pool(name="y", bufs=1))
                y2T_pool = ffn.enter_context(tc.tile_pool(name="y2T", bufs=1))
                g_pool = ffn.enter_context(tc.tile_pool(name="g", bufs=64))
                yw = ffn.enter_context(tc.tile_pool(name="yw", bufs=3))
                lnst2 = ffn.enter_context(tc.tile_pool(name="lnst2", bufs=4))
                w1_pool = ffn.enter_context(tc.tile_pool(name="w1", bufs=5))
                w2_pool = ffn.enter_context(tc.tile_pool(name="w2", bufs=5))
                out_pool = ffn.enter_context(tc.tile_pool(name="outp", bufs=3))
                ff_ps_pool = ffn.enter_context(tc.tile_pool(name="ff_ps", bufs=2, space="PSUM"))
                z_ps_pool = ffn.enter_context(tc.tile_pool(name="z_ps", bufs=1, space="PSUM"))

                y = [y_pool.tile([NP, D], F32, tag=f"y{rc}", name=f"y{rc}") for rc in range(8)]
                y2T = y2T_pool.tile([NP, DC, RROWS], BF16)

                cc_outs = (cc_out0, cc_out1)
                for half in range(2):
                    # residual + LN2 for this half only, so half 0's FFN can
                    # run while the second ReduceScatter is still in flight
                    for r4 in range(4):
                        rc = half * 4 + r4
                        rs_sb = yw.tile([NP, D], BF16, tag="rs")
                        for st in range(2):
                            nc.sync.dma_start(
                                out=rs_sb[:, st * 512:(st + 1) * 512],
                                in_=cc_outs[half][r4 * NP:(r4 + 1) * NP, st * 512:(st + 1) * 512])
                        xr_sb = yw.tile([NP, D], F32, tag="xr")
                        nc.sync.dma_start(out=xr_sb[:], in_=xres_p[rc * NP:(rc + 1) * NP, :])
                        nc.vector.tensor_copy(out=y[rc][:], in_=rs_sb[:])
                        nc.vector.tensor_add(out=y[rc][:], in0=y[rc][:], in1=xr_sb[:])
                        nc.vector.tensor_add(out=y[rc][:], in0=y[rc][:], in1=bproj_bc[:])
                        mv = _ln_stats(nc, lnst2, y[rc][:], eps_sb)
                        y2_sb = yw.tile([NP, D], BF16, tag="y2")
                        nc.vector.tensor_scalar(
                            out=y2_sb[:], in0=y[rc][:],
                            scalar1=mv[:, 0:1], scalar2=mv[:, 1:2],
                            op0=OP.subtract, op1=OP.mult)
                        if apply_ln2_affine:
                            nc.vector.tensor_mul(out=y2_sb[:], in0=y2_sb[:], in1=ln2w_bc[:])
                            nc.vector.tensor_add(out=y2_sb[:], in0=y2_sb[:], in1=ln2b_bc[:])
                        for dc in range(DC):
                            t_ps = tp_ps.tile([NP, NP], BF16, tag="tp")
                            nc.tensor.transpose(t_ps[:], y2_sb[:, dc * NP:(dc + 1) * NP], ident_sb[:])
                            nc.scalar.copy(out=y2T[:, dc, rc * NP:(rc + 1) * NP], in_=t_ps[:])

                    hcols = slice(half * 512, (half + 1) * 512)
                    gts = []
                    for ffc in range(FC):
                        w1_sb = w1_pool.tile([NP, DC * NP], BF16, tag="w1")
                        nc.sync.dma_start(out=w1_sb[:], in_=wff1_p[ffc, :, :])
                        ff_ps = ff_ps_pool.tile([NP, 512], F32, tag="ffps")
                        for dc in range(DC):
                            nc.tensor.matmul(
                                ff_ps[:],
                                w1_sb[:, dc * NP:(dc + 1) * NP],
                                y2T[:, dc, hcols],
                                start=(dc == 0), stop=(dc == DC - 1))
                        g_sb = g_pool.tile([NP, 512], BF16, tag="g")
                        nc.scalar.activation(out=g_sb[:], in_=ff_ps[:], func=AF.Gelu,
                                             bias=bff1_sb[:, ffc:ffc + 1], scale=1.0)
                        gts.append(g_sb)
                    for nch in range(2):
                        ncols = slice(nch * 512, (nch + 1) * 512)
                        z_pss = [z_ps_pool.tile([NP, 512], F32, tag=f"zps{r}", name=f"zps{r}") for r in range(4)]
                        for ffc in range(FC):
                            w2_sb = w2_pool.tile([NP, 512], BF16, tag="w2")
                            nc.sync.dma_start(out=w2_sb[:], in_=wff2_p[ffc, :, ncols])
                            for r4 in range(4):
                                nc.tensor.matmul(
                                    z_pss[r4][:],
                                    gts[ffc][:, r4 * NP:(r4 + 1) * NP],
                                    w2_sb[:],
                                    start=(ffc == 0), stop=(ffc == FC - 1))
                        for r4 in range(4):
                            rc = half * 4 + r4
                            o_sb = out_pool.tile([NP, 512], F32, tag="osb")
                            nc.vector.tensor_add(out=o_sb[:], in0=z_pss[r4][:], in1=y[rc][:, ncols])
                            nc.vector.tensor_add(out=o_sb[:], in0=o_sb[:], in1=bff2_bc[:, ncols])
                            nc.sync.dma_start(out=out_p[rc * NP:(rc + 1) * NP, ncols], in_=o_sb[:])

    nc.compile()
    return nc


# ------------------------- host-side driver -------------------------

_BF = ml_dtypes.bfloat16


def _core_rows(g):
    return np.r_[512 * g:512 * g + 512, 1024 + 512 * g:1536 + 512 * g]


def _prep_core_inputs(inputs, b, g):
    x = np.asarray(inputs["x"], np.float32)
    w_qkv = np.asarray(inputs["w_qkv"], np.float32).reshape(D, H, HD, 3)
    hs = slice(g * GH, (g + 1) * GH)
    w_k = w_qkv[:, hs, :, 0].reshape(D, GH * HD)
    w_q = (w_qkv[:, hs, :, 1] * (HD ** -0.5)).reshape(D, GH * HD)
    w_v = w_qkv[:, hs, :, 2].reshape(D, GH * HD)

    def tile_kxm(w):  # [D, 512] -> [128, DC, 512]
        return np.ascontiguousarray(w.reshape(DC, NP, GH * HD).transpose(1, 0, 2))

    w_proj = np.asarray(inputs["w_proj"], np.float32)
    wp = np.ascontiguousarray(
        w_proj[g * 512:(g + 1) * 512, :].reshape(4, NP, D).transpose(1, 0, 2))

    w_ff1 = np.asarray(inputs["w_ff1"], np.float32)
    w1t = np.ascontiguousarray(
        w_ff1.reshape(DC, NP, FC, NP).transpose(2, 1, 0, 3).reshape(FC, NP, DC * NP))
    w_ff2 = np.asarray(inputs["w_ff2"], np.float32)
    w2t = np.ascontiguousarray(w_ff2.reshape(FC, NP, D))

    j = np.arange(NP)[:, None]
    i = np.arange(NP)[None, :]
    mask = (j <= i).astype(np.float32)

    return {
        "x": np.ascontiguousarray(x[b]),
        "x_res": np.ascontiguousarray(x[b][_core_rows(g)]),
        "wq": tile_kxm(w_q).astype(_BF),
        "wk": tile_kxm(w_k).astype(_BF),
        "wv": tile_kxm(w_v).astype(_BF),
        "wproj": wp.astype(_BF),
        "wff1": w1t.astype(_BF),
        "wff2": w2t.astype(_BF),
        "bproj": np.asarray(inputs["b_proj"], np.float32).reshape(1, D).copy(),
        "bff1": np.ascontiguousarray(np.asarray(inputs["b_ff1"], np.float32).reshape(FC, NP).T),
        "bff2": np.asarray(inputs["b_ff2"], np.float32).reshape(1, D).copy(),
        "ident": np.eye(NP, dtype=_BF),
        "mask": mask.astype(_BF),
    }


_NC_CACHE = {}


def kernel(**inputs):
    ln1w = np.asarray(inputs["ln1_w"], np.float32)
    ln1b = np.asarray(inputs["ln1_b"], np.float32)
    ln2w = np.asarray(inputs["ln2_w"], np.float32)
    ln2b = np.asarray(inputs["ln2_b"], np.float32)
    a1 = not (np.allclose(ln1w, 1.0) and np.allclose(ln1b, 0.0))
    a2 = not (np.allclose(ln2w, 1.0) and np.allclose(ln2b, 0.0))

    key = (a1, a2)
    if key not in _NC_CACHE:
        _NC_CACHE[key] = build(apply_ln1_affine=a1, apply_ln2_affine=a2)
    nc = _NC_CACHE[key]

    in_maps = []
    for core in range(8):
        b, g = core // 2, core % 2
        m = _prep_core_inputs(inputs, b, g)
        if a1:
            m["ln1w"] = ln1w.reshape(1, D).copy()
            m["ln1b"] = ln1b.reshape(1, D).copy()
        if a2:
            m["ln2w"] = ln2w.reshape(1, D).copy()
            m["ln2b"] = ln2b.reshape(1, D).copy()
        in_maps.append(m)

    res = run_bass_kernel_spmd(nc, in_maps, core_ids=list(range(8)))

    out = np.empty((B, S, D), np.float32)
    for core in range(8):
        b, g = core // 2, core % 2
        out[b][_core_rows(g)] = res.results[core]["out"]
    return out
